# revision 1
# baseline (speedup 1.0000x reference)
"""DeltaNet fused kernel for 8 TRN2 NeuronCores (Bass/Tile).

Math (reference):
    s  = x @ W_slow_w.T + W_slow_b            [B, 3073]
    k  = s[:, :1024]; v = s[:, 1024:2048]; q = s[:, 2048:3072]
    lr = sigmoid(s[:, 3072])
    v_bar = softmax(k) @ W_fast_w.T + W_fast_b          (W_fast_w == 0 -> v_bar = W_fast_b)
    deltaT[h,o] = sum_b sigmoid(k)[b,h] * (lr*(v - v_bar))[b,o] / B
    out = softmax(q) @ (W_fast_w + delta).T + W_fast_b

Sharding: data-parallel over batch (2048 rows/core). deltaT partials are
AllReduced across the 8 cores (two AllReduces, one per batch half, so the
first overlaps the second half's compute).

Device layout trick: all matmuls use natural layouts (no on-chip transposes):
  - host pre-transposes x (per-shard) and W_slow_w to bf16
  - q is computed TRANSPOSED (qT[h,b]) so exp(qT) is directly the lhsT of the
    final matmul; softmax denominator comes from a ones-vector matmul column
    and is applied as a per-partition output scale.
"""

import os
import sys

for _p in ("/opt/trn_rl_repo", "/root/.axon_site/_ro/trn_rl_repo"):
    if os.path.isdir(_p) and _p not in sys.path:
        sys.path.append(_p)

import numpy as np
import ml_dtypes

BF16 = ml_dtypes.bfloat16

N_CORES = 8
B_FULL = 16384
DIM = 1024          # dim_in == dim_out == dim_hidden
SLOW_OUT = 3 * DIM + 1
P = 128
NT = DIM // P       # 8 tiles along any 1024 dim


def _build_program(b_core: int, n_cores: int = N_CORES):
    """Build the SPMD Bass program (same program on every core)."""
    import concourse.bass as bass
    import concourse.mybir as mybir
    import concourse.tile as tile
    from concourse import bacc

    f32 = mybir.dt.float32
    bf16 = mybir.dt.bfloat16
    AF = mybir.ActivationFunctionType
    ALU = mybir.AluOpType

    assert b_core % (2 * P) == 0
    nbt = b_core // P                 # b-tiles per core
    # asymmetric halves: the larger second half's q-phase hides the second
    # AllReduce's ~50us cost
    nbt1 = max(2, nbt // 4)
    nbt2 = nbt - nbt1
    bh = [nbt1 * P, nbt2 * P]
    off = [0, nbt1 * P]
    nbt_h = [nbt1, nbt2]

    def chunks(width):
        lo = 0
        while lo < width:
            hi = min(lo + 512, width)
            yield lo, hi
            lo = hi

    nc = bacc.Bacc(
        "TRN2",
        target_bir_lowering=False,
        debug=False,
        num_devices=n_cores,
    )

    # ---- kernel I/O ----
    xT_h = nc.dram_tensor("xT", [DIM, b_core], bf16, kind="ExternalInput")
    wT_h = nc.dram_tensor("wT", [DIM, SLOW_OUT], bf16, kind="ExternalInput")
    bk_h = nc.dram_tensor("bk", [DIM], f32, kind="ExternalInput")     # W_slow_b[:1024]
    bvc_h = nc.dram_tensor("bvc", [DIM], f32, kind="ExternalInput")   # W_slow_b[1024:2048] - W_fast_b
    bq_h = nc.dram_tensor("bq", [DIM], f32, kind="ExternalInput")     # W_slow_b[2048:3072]
    blr_h = nc.dram_tensor("blr", [1], f32, kind="ExternalInput")     # W_slow_b[3072]
    wfb_h = nc.dram_tensor("wfb", [DIM], f32, kind="ExternalInput")   # W_fast_b
    out_h = nc.dram_tensor("out", [b_core, DIM], f32, kind="ExternalOutput")

    inv_b = 1.0 / float(b_core * n_cores)

    with tile.TileContext(nc) as tc:
        with (
            tc.tile_pool(name="persist", bufs=1) as persist,
            tc.tile_pool(name="psum", bufs=8, space="PSUM") as psum,
            tc.tile_pool(name="tmp", bufs=4) as tmp,
            tc.tile_pool(name="small", bufs=6) as small,
            tc.tile_pool(name="ost", bufs=2) as ost,
            tc.tile_pool(name="dst", bufs=2) as dstp,
            tc.tile_pool(name="arl", bufs=2) as arl,
            tc.tile_pool(name="dram", bufs=1, space="DRAM") as dram,
        ):
            # ---- persistent SBUF tensors ----
            # wT split into separate tiles per column group so the first
            # s-matmuls only depend on their own group's loads (Tile tracks
            # dependencies per tile object)
            wk = [persist.tile([P, NT, 512], bf16, name=f"wk{c}") for c in range(2)]
            wv = [persist.tile([P, NT, 512], bf16, name=f"wv{c}") for c in range(2)]
            wlr = persist.tile([P, NT, 1], bf16, name="wlr")
            wq = persist.tile([P, NT, DIM], bf16, name="wq")
            xt = persist.tile([P, NT, max(bh)], bf16, name="xt")     # one half at a time
            et = persist.tile([P, NT, b_core], bf16, name="et")      # exp(qT), full batch
            sigk = persist.tile([P, max(nbt_h), DIM], bf16, name="sigk")
            u = persist.tile([P, max(nbt_h), DIM], bf16, name="u")
            wn = [persist.tile([P, DIM], bf16, name=f"wn{i}") for i in range(NT)]
            bk_b = persist.tile([P, DIM], f32, name="bk_b")
            bvc_b = persist.tile([P, DIM], f32, name="bvc_b")
            wfb_b = persist.tile([P, DIM], f32, name="wfb_b")
            bq_c = persist.tile([P, NT], f32, name="bq_c")
            blr_c = persist.tile([P, 1], f32, name="blr_c")
            ones = persist.tile([P, 1], bf16, name="ones")

            # ---- DRAM bounce buffers for the two AllReduces ----
            ar_in = [
                dram.tile([DIM, DIM], bf16, name=f"ar_in{h}") for h in range(2)
            ]
            ar_out = [
                dram.tile([DIM, DIM], bf16, name=f"ar_out{h}", addr_space="Shared")
                for h in range(2)
            ]

            # ---- constants / weights ----
            nc.vector.memset(ones[:], 1.0)
            # broadcast biases across partitions via K=1 ones-matmuls (PE is
            # idle at startup; saves 1.5 MiB of broadcast-DMA in the critical
            # startup window)
            ones_row = persist.tile([1, P], f32, name="ones_row")
            nc.vector.memset(ones_row[:], 1.0)
            for bi, (bias_dst, bias_src) in enumerate(
                ((bk_b, bk_h), (bvc_b, bvc_h), (wfb_b, wfb_h))
            ):
                for c in range(2):
                    brow = tmp.tile([1, 512], f32, tag="kv", name=f"br{bi}_{c}")
                    nc.gpsimd.dma_start(
                        out=brow[:],
                        in_=bass.AP(tensor=bias_src, offset=c * 512,
                                    ap=[[0, 1], [1, 512]]),
                    )
                    pb = psum.tile([P, 512], f32, tag="ps", name=f"pb{bi}_{c}")
                    nc.tensor.matmul(
                        pb[:], ones_row[:, :], brow[:], start=True, stop=True
                    )
                    nc.vector.tensor_copy(bias_dst[:, c * 512:(c + 1) * 512], pb[:])
            # bq_c[p, i] = bq[i*128 + p]
            nc.gpsimd.dma_start(
                out=bq_c[:],
                in_=bass.AP(tensor=bq_h, offset=0, ap=[[1, P], [P, NT]]),
            )
            nc.gpsimd.dma_start(
                out=blr_c[:],
                in_=bass.AP(tensor=blr_h, offset=0, ap=[[0, P], [1, 1]]),
            )
            # xT (half 0) first so the s-phase can start ASAP, then wT groups
            # in consumption order (lr column first: tiny).
            for i in range(NT):
                nc.sync.dma_start(
                    out=xt[:, i, 0:bh[0]], in_=xT_h[i * P:(i + 1) * P, 0:bh[0]]
                )
            for c in range(2):
                for i in range(NT):
                    nc.sync.dma_start(
                        out=wk[c][:, i, :],
                        in_=wT_h[i * P:(i + 1) * P, c * 512:(c + 1) * 512],
                    )
            for i in range(NT):
                nc.sync.dma_start(
                    out=wlr[:, i, :], in_=wT_h[i * P:(i + 1) * P, 3 * DIM:SLOW_OUT]
                )
            for c in range(2):
                for i in range(NT):
                    nc.sync.dma_start(
                        out=wv[c][:, i, :],
                        in_=wT_h[i * P:(i + 1) * P, DIM + c * 512:DIM + (c + 1) * 512],
                    )
            for i in range(NT):
                nc.sync.dma_start(
                    out=wq[:, i, :], in_=wT_h[i * P:(i + 1) * P, 2 * DIM:3 * DIM]
                )

            def drain_lr(half, t, plr):
                # lr = sigmoid(plr + blr) / B
                lr_s = small.tile([P, 1], f32, tag="lr", name=f"lr{half}_{t}")
                nc.scalar.activation(lr_s[:], plr[:], AF.Sigmoid, bias=blr_c[:, 0:1])
                nc.vector.tensor_scalar_mul(lr_s[:], lr_s[:], inv_b)
                return lr_s

            def drain_k(half, t, c, ps_c):
                # sigk = sigmoid(k + bk)
                ktmp = tmp.tile([P, 512], f32, tag="kv", name=f"kt{half}_{t}_{c}")
                nc.vector.tensor_add(ktmp[:], ps_c[:], bk_b[:, c * 512:(c + 1) * 512])
                nc.scalar.activation(
                    sigk[:, t, c * 512:(c + 1) * 512], ktmp[:], AF.Sigmoid
                )

            def drain_v(half, t, c, ps_c, lr_s):
                # u = lr/B * (v + (bv - wfb))
                vtmp = tmp.tile([P, 512], f32, tag="kv", name=f"vt{half}_{t}_{c}")
                nc.vector.tensor_add(
                    vtmp[:], ps_c[:], bvc_b[:, c * 512:(c + 1) * 512]
                )
                nc.scalar.activation(
                    u[:, t, c * 512:(c + 1) * 512], vtmp[:], AF.Copy,
                    scale=lr_s[:],
                )

            def emit_s_early(n_early):
                """First b-tiles of half 0 in chunk-major order so consumption
                matches the wT groups' DMA arrival order (keeps every PE wait
                under the ~3.4us HAM re-throttle window at startup)."""
                lrs = []
                for t in range(n_early):
                    plr = psum.tile([P, 1], f32, tag="ps", name=f"plrE_{t}")
                    for i in range(NT):
                        nc.tensor.matmul(
                            plr[:], xt[:, i, t * P:(t + 1) * P], wlr[:, i, :],
                            start=(i == 0), stop=(i == NT - 1),
                        )
                    lrs.append(drain_lr(0, t, plr))
                for c in range(4):
                    w_t = wk[c] if c < 2 else wv[c - 2]
                    for t in range(n_early):
                        ps_c = psum.tile([P, 512], f32, tag="ps", name=f"psE_{t}_{c}")
                        for i in range(NT):
                            nc.tensor.matmul(
                                ps_c[:], xt[:, i, t * P:(t + 1) * P], w_t[:, i, :],
                                start=(i == 0), stop=(i == NT - 1),
                            )
                        if c < 2:
                            drain_k(0, t, c, ps_c)
                        else:
                            drain_v(0, t, c - 2, ps_c, lrs[t])

            def emit_s(half, t0=0):
                """k / v / lr per b-tile; fills sigk and u for this half."""
                for t in range(t0, nbt_h[half]):
                    ps = [
                        psum.tile([P, 512], f32, tag="ps", name=f"ps{half}_{t}_{c}")
                        for c in range(4)
                    ]
                    plr = psum.tile([P, 1], f32, tag="ps", name=f"plr{half}_{t}")
                    for i in range(NT):
                        lhs = xt[:, i, t * P:(t + 1) * P]
                        st = dict(start=(i == 0), stop=(i == NT - 1))
                        for c in range(2):
                            nc.tensor.matmul(ps[c][:], lhs, wk[c][:, i, :], **st)
                        for c in range(2):
                            nc.tensor.matmul(ps[2 + c][:], lhs, wv[c][:, i, :], **st)
                        nc.tensor.matmul(plr[:], lhs, wlr[:, i, :], **st)
                    lr_s = drain_lr(half, t, plr)
                    for c in range(2):
                        drain_k(half, t, c, ps[c])
                        drain_v(half, t, c, ps[2 + c], lr_s)

            def emit_delta(half):
                """deltaT_half[h, o] = sum_b sigk * u, staged + AllReduced.

                For the second half the first AllReduce's output is folded
                into the input (scaled by 1/n_cores so the sum over cores
                adds it exactly once) — W_newT is then just ar_out[1]."""
                nb = nbt_h[half]
                for hh in range(NT):
                    pd = [
                        psum.tile([P, 512], f32, tag="ps", name=f"pd{half}_{hh}_{oc}")
                        for oc in range(2)
                    ]
                    for t in range(nb):
                        st = dict(start=(t == 0), stop=(t == nb - 1))
                        lhs = sigk[:, t, hh * P:(hh + 1) * P]
                        for oc in range(2):
                            nc.tensor.matmul(
                                pd[oc][:], lhs, u[:, t, oc * 512:(oc + 1) * 512], **st
                            )
                    dstage = dstp.tile([P, DIM], bf16, tag="ds", name=f"ds{half}_{hh}")
                    if half == 0:
                        for oc in range(2):
                            nc.vector.tensor_copy(
                                dstage[:, oc * 512:(oc + 1) * 512], pd[oc][:]
                            )
                    else:
                        a0 = arl.tile([P, DIM], bf16, tag="ar", name=f"a0_{hh}")
                        nc.sync.dma_start(
                            out=a0[:], in_=ar_out[0][hh * P:(hh + 1) * P, :]
                        )
                        for oc in range(2):
                            nc.vector.scalar_tensor_tensor(
                                dstage[:, oc * 512:(oc + 1) * 512],
                                a0[:, oc * 512:(oc + 1) * 512],
                                1.0 / n_cores,
                                pd[oc][:],
                                op0=ALU.mult,
                                op1=ALU.add,
                            )
                    nc.sync.dma_start(
                        out=ar_in[half][hh * P:(hh + 1) * P, :], in_=dstage[:]
                    )
                nc.gpsimd.collective_compute(
                    "AllReduce",
                    mybir.AluOpType.add,
                    replica_groups=[list(range(n_cores))],
                    ins=[ar_in[half][:, :]],
                    outs=[ar_out[half][:, :]],
                )
                if half == 1:
                    for hh in range(NT):
                        nc.sync.dma_start(
                            out=wn[hh][:], in_=ar_out[1][hh * P:(hh + 1) * P, :]
                        )

            def emit_q(half):
                """et = exp(qT + bq) (transposed layout)."""
                for hh in range(NT):
                    for lo, hi in chunks(bh[half]):
                        w = hi - lo
                        pq = psum.tile(
                            [P, 512], f32, tag="ps", name=f"pq{half}_{hh}_{lo}"
                        )
                        for i in range(NT):
                            nc.tensor.matmul(
                                pq[:, 0:w],
                                wq[:, i, hh * P:(hh + 1) * P],
                                xt[:, i, lo:hi],
                                start=(i == 0),
                                stop=(i == NT - 1),
                            )
                        nc.scalar.activation(
                            et[:, hh, off[half] + lo:off[half] + hi],
                            pq[:, 0:w],
                            AF.Exp,
                            bias=bq_c[:, hh:hh + 1],
                        )

            # half 0: q before delta so the delta matmuls cover the xT reload;
            # half 1: delta first so AR2 overlaps the (large) q2-phase.
            n_early = min(2, nbt_h[0])
            emit_s_early(n_early)
            emit_s(0, t0=n_early)
            emit_q(0)
            emit_delta(0)
            for i in range(NT):
                nc.sync.dma_start(
                    out=xt[:, i, 0:bh[1]],
                    in_=xT_h[i * P:(i + 1) * P, off[1]:off[1] + bh[1]],
                )
            emit_s(1)
            emit_delta(1)
            emit_q(1)

            # ---- final: out = (et.T @ wn) / rowsum + wfb ----
            for t in range(nbt):
                po = [
                    psum.tile([P, 512], f32, tag="ps", name=f"po{t}_{oc}")
                    for oc in range(2)
                ]
                prs = psum.tile([P, 1], f32, tag="ps", name=f"prs{t}")
                for hh in range(NT):
                    lhs = et[:, hh, t * P:(t + 1) * P]
                    st = dict(start=(hh == 0), stop=(hh == NT - 1))
                    for oc in range(2):
                        nc.tensor.matmul(
                            po[oc][:], lhs, wn[hh][:, oc * 512:(oc + 1) * 512], **st
                        )
                    nc.tensor.matmul(prs[:], lhs, ones[:], **st)
                recip = small.tile([P, 1], f32, tag="rc", name=f"rc{t}")
                nc.vector.reciprocal(recip[:], prs[:])
                o_st = ost.tile([P, DIM], f32, tag="os", name=f"os{t}")
                for oc in range(2):
                    nc.vector.scalar_tensor_tensor(
                        o_st[:, oc * 512:(oc + 1) * 512],
                        po[oc][:],
                        recip[:],
                        wfb_b[:, oc * 512:(oc + 1) * 512],
                        op0=ALU.mult,
                        op1=ALU.add,
                    )
                nc.sync.dma_start(out=out_h[t * P:(t + 1) * P, :], in_=o_st[:])

    nc.compile()
    return nc


def _host_prep(x, W_slow_w, W_slow_b, W_fast_b, b_core, n_cores):
    """Shard + pre-transpose + cast inputs; returns per-core input maps."""
    wT = np.ascontiguousarray(W_slow_w.T).astype(BF16)
    bk = np.ascontiguousarray(W_slow_b[:DIM]).astype(np.float32)
    bvc = (W_slow_b[DIM:2 * DIM] - W_fast_b).astype(np.float32)
    bq = np.ascontiguousarray(W_slow_b[2 * DIM:3 * DIM]).astype(np.float32)
    blr = np.ascontiguousarray(W_slow_b[3 * DIM:3 * DIM + 1]).astype(np.float32)
    wfb = np.ascontiguousarray(W_fast_b).astype(np.float32)
    in_maps = []
    for c in range(n_cores):
        xs = x[c * b_core:(c + 1) * b_core, :]
        xT = np.ascontiguousarray(xs.T).astype(BF16)
        in_maps.append(
            {"xT": xT, "wT": wT, "bk": bk, "bvc": bvc, "bq": bq, "blr": blr,
             "wfb": wfb}
        )
    return in_maps


_PROGRAM_CACHE = {}


def _get_program(b_core, n_cores=N_CORES):
    key = (b_core, n_cores)
    if key not in _PROGRAM_CACHE:
        _PROGRAM_CACHE[key] = _build_program(b_core, n_cores)
    return _PROGRAM_CACHE[key]


def _run_device(x, W_slow_w, W_slow_b, W_fast_b, trace=False):
    from concourse.bass_utils import run_bass_kernel_spmd

    b_core = x.shape[0] // N_CORES
    nc = _get_program(b_core)
    in_maps = _host_prep(x, W_slow_w, W_slow_b, W_fast_b, b_core, N_CORES)
    res = run_bass_kernel_spmd(nc, in_maps, list(range(N_CORES)), trace=trace)
    out = np.concatenate([res.results[c]["out"] for c in range(N_CORES)], axis=0)
    return out.astype(np.float32), res


def _reference_numpy(x, W_slow_w, W_slow_b, W_fast_w, W_fast_b):
    """Exact fallback (only used if W_fast_w != 0, which the spec never produces)."""
    x = x.astype(np.float64)
    s = x @ W_slow_w.astype(np.float64).T + W_slow_b.astype(np.float64)
    k = s[:, :DIM]
    v = s[:, DIM:2 * DIM]
    q = s[:, 2 * DIM:3 * DIM]
    lr = 1.0 / (1.0 + np.exp(-s[:, -1:]))
    ek = np.exp(k - k.max(axis=1, keepdims=True))
    ak = ek / ek.sum(axis=1, keepdims=True)
    v_bar = ak @ W_fast_w.astype(np.float64).T + W_fast_b.astype(np.float64)
    sigk = 1.0 / (1.0 + np.exp(-k))
    delta = (lr * (v - v_bar)).T @ sigk / x.shape[0]
    w_new = W_fast_w.astype(np.float64) + delta
    eq = np.exp(q - q.max(axis=1, keepdims=True))
    aq = eq / eq.sum(axis=1, keepdims=True)
    return (aq @ w_new.T + W_fast_b.astype(np.float64)).astype(np.float32)


def kernel(x, W_slow_w, W_slow_b, W_fast_w, W_fast_b):
    x = np.asarray(x)
    W_slow_w = np.asarray(W_slow_w)
    W_slow_b = np.asarray(W_slow_b)
    W_fast_w = np.asarray(W_fast_w)
    W_fast_b = np.asarray(W_fast_b)
    if np.any(W_fast_w):
        # Spec guarantees W_fast_w == 0; exact fallback for generality.
        return _reference_numpy(x, W_slow_w, W_slow_b, W_fast_w, W_fast_b)
    out, _ = _run_device(x, W_slow_w, W_slow_b, W_fast_b, trace=False)
    return out



# revision 10
# speedup vs baseline: 1.0993x; 1.0993x over previous
"""DeltaNet fused kernel for 8 TRN2 NeuronCores (Bass/Tile), fp8-hybrid.

Math (reference, with W_fast_w == 0 so v_bar == W_fast_b):
    s  = x @ W_slow_w.T + W_slow_b            [B, 3073]
    k  = s[:, :1024]; v = s[:, 1024:2048]; q = s[:, 2048:3072]
    lr = sigmoid(s[:, 3072])
    delta[o,h] = sum_b (lr*(v - wfb))[b,o] * sigmoid(k)[b,h] / B
    out = softmax(q) @ delta.T + wfb

Restructured to eliminate the v projection (v = x @ Wv.T + bv):
    g  = lr * sigmoid(k)                      [B, H]
    M  = x.T @ g                              [I, H]   (AllReduced partial sums)
    r  = sum_b g[b, :]                        [H]
    delta.T = (M.T @ Wv.T + r x (bv - wfb)) / B        [H, O]
    out = softmax(q) @ delta.T + wfb

This removes the [B,O] v matmul entirely and shrinks the batch-sized work
from 21.5 to 17.2 GFLOP/core; the Wv contraction runs once per core on the
small [I,H] M instead of per row.

Precision (chosen by numpy simulation of quantization error; tolerance 2e-2):
    fp8-e4m3 + DoubleRow (~1.44x bf16):  q-matmul, M-matmul, final matmul
    bf16:                                k-matmul, lr, delta.T (pd) matmul
    AllReduce in fp8 (scale 4096): its output is directly the final rhs.
  Simulated end-to-end rel err 6.6e-3 (bf16 everywhere: 1.4e-3).

Sharding: data-parallel over batch (2048 rows/core); one fp8 AllReduce of
the [H,O] delta partials, overlapped with the second half of the q-phase.
"""

import os
import sys

for _p in ("/opt/trn_rl_repo", "/root/.axon_site/_ro/trn_rl_repo"):
    if os.path.isdir(_p) and _p not in sys.path:
        sys.path.append(_p)

import numpy as np
import ml_dtypes

BF16 = ml_dtypes.bfloat16
F8E4 = ml_dtypes.float8_e4m3     # TRN fp8e4: max normal +-240

N_CORES = 8
B_FULL = 16384
DIM = 1024          # dim_in == dim_out == dim_hidden
P = 128
NT = DIM // P       # 8 tiles along any 1024 dim

SX = 16.0           # x fp8 scale           (|x|max 5.4  -> 87)
SWQ = 512.0         # Wq fp8 scale          (|w|max .16  -> 80)
SG = 128.0          # g fp8 scale           (g in (0,1)  -> <128)
SAR = 4096.0        # delta fp8 scale       (|delta|max .043 -> 176)
SHIFT = 3.0         # exp shift             (max q+bq 7.63 -> et < 103)


def _build_program(b_core: int, n_cores: int = N_CORES):
    """Build the SPMD Bass program (same program on every core)."""
    import concourse.bass as bass
    import concourse.mybir as mybir
    import concourse.tile as tile
    from concourse import bacc

    f32 = mybir.dt.float32
    bf16 = mybir.dt.bfloat16
    f8 = mybir.dt.float8e4
    AF = mybir.ActivationFunctionType
    ALU = mybir.AluOpType
    DR = mybir.MatmulPerfMode.DoubleRow

    nbt = b_core // P               # b-tiles per core (16)
    nbp = nbt // 2                  # b-pairs for DoubleRow contraction (8)
    nbc = b_core // 512             # 512-wide b-chunks (4)
    assert b_core % 1024 == 0

    nc = bacc.Bacc(
        "TRN2",
        target_bir_lowering=False,
        debug=False,
        num_devices=n_cores,
    )

    # ---- kernel I/O ----
    xT16_h = nc.dram_tensor("xT16", [DIM, b_core], bf16, kind="ExternalInput")
    xT8_h = nc.dram_tensor("xT8", [DIM, b_core], f8, kind="ExternalInput")
    xn8_h = nc.dram_tensor("xn8", [b_core, DIM], f8, kind="ExternalInput")
    wk16_h = nc.dram_tensor("wk16", [DIM, DIM], bf16, kind="ExternalInput")
    wq8_h = nc.dram_tensor("wq8", [DIM, DIM], f8, kind="ExternalInput")
    wv16_h = nc.dram_tensor("wv16", [DIM, DIM], bf16, kind="ExternalInput")
    wlr16_h = nc.dram_tensor("wlr16", [DIM], bf16, kind="ExternalInput")
    bk_h = nc.dram_tensor("bk", [DIM], f32, kind="ExternalInput")
    bqs_h = nc.dram_tensor("bqs", [DIM], f32, kind="ExternalInput")   # bq - SHIFT
    blr_h = nc.dram_tensor("blr", [1], f32, kind="ExternalInput")
    # (bv - wfb) * SAR / (b_total * SG): outer-product pre-scaled for the drain
    bvcp_h = nc.dram_tensor("bvcp", [DIM], f32, kind="ExternalInput")
    wfb_h = nc.dram_tensor("wfb", [DIM], f32, kind="ExternalInput")
    out_h = nc.dram_tensor("out", [b_core, DIM], f32, kind="ExternalOutput")

    b_total = float(b_core * n_cores)
    # fold every static scale into the drain constants
    pd_drain_scale = SAR / (b_total * SX * SG)
    fin_recip_scale = 1.0 / SAR
    q_act_scale = 1.0 / (SX * SWQ)

    with tile.TileContext(nc) as tc:
        with (
            tc.tile_pool(name="persist", bufs=1) as persist,
            tc.tile_pool(name="psum", bufs=6, space="PSUM") as psum,
            tc.tile_pool(name="psmall", bufs=2, space="PSUM") as psmall,
            tc.tile_pool(name="tmp", bufs=4) as tmp,
            tc.tile_pool(name="small", bufs=6) as small,
            tc.tile_pool(name="ost", bufs=2) as ost,
            tc.tile_pool(name="arst", bufs=2) as arst,
            tc.tile_pool(name="dram", bufs=1, space="DRAM") as dram,
        ):
            # ---- persistent SBUF tensors ----
            wq8 = persist.tile([P, NT, DIM], f8, name="wq8")
            xT8a = persist.tile([P, NT, b_core // 2], f8, name="xT8a")
            xT8b = persist.tile([P, NT, b_core // 2], f8, name="xT8b")
            wk16 = [persist.tile([P, NT, 512], bf16, name=f"wk16_{c}") for c in range(2)]
            wlr16 = persist.tile([P, NT, 1], bf16, name="wlr16")
            xT16 = persist.tile([P, NT, b_core], bf16, name="xT16")
            xn8 = persist.tile([P, nbt, DIM], f8, name="xn8")
            wv16 = persist.tile([P, NT, DIM], bf16, name="wv16")
            g8 = persist.tile([P, nbt, DIM], f8, name="g8")
            et8 = persist.tile([P, NT, b_core], f8, name="et8")
            mb = persist.tile([P, NT, DIM], bf16, name="mb")
            wn8 = persist.tile([P, NT, DIM], f8, name="wn8")
            bk_b = persist.tile([P, DIM], f32, name="bk_b")
            wfb_b = persist.tile([P, DIM], f32, name="wfb_b")
            bvcp_b = persist.tile([P, DIM], f32, name="bvcp_b")
            bq_c = persist.tile([P, NT], f32, name="bq_c")
            blr_c = persist.tile([P, 1], f32, name="blr_c")
            r_c = persist.tile([P, NT], f32, name="r_c")
            ones8 = persist.tile([P, 2, 16], f8, name="ones8")
            ones_row = persist.tile([1, P], f32, name="ones_row")

            # ---- DRAM bounce buffers for the AllReduce ----
            ar_in = dram.tile([DIM, DIM], bf16, name="ar_in")
            ar_out = dram.tile([DIM, DIM], bf16, name="ar_out", addr_space="Shared")

            nc.vector.memset(ones8[:], 1.0)
            nc.vector.memset(ones_row[:], 1.0)

            # ---- small DMAs first (needed by early drains) ----
            nc.gpsimd.dma_start(
                out=bq_c[:],
                in_=bass.AP(tensor=bqs_h, offset=0, ap=[[1, P], [P, NT]]),
            )
            nc.gpsimd.dma_start(
                out=blr_c[:],
                in_=bass.AP(tensor=blr_h, offset=0, ap=[[0, P], [1, 1]]),
            )
            for i in range(NT):
                nc.gpsimd.dma_start(
                    out=wlr16[:, i, :],
                    in_=bass.AP(tensor=wlr16_h, offset=i * P, ap=[[1, P], [P, 1]]),
                )
            # bias broadcasts across partitions via K=1 ones-matmuls
            for bi, (bias_dst, bias_src) in enumerate(
                ((bk_b, bk_h), (wfb_b, wfb_h), (bvcp_b, bvcp_h))
            ):
                for c in range(2):
                    brow = tmp.tile([1, 512], f32, tag="kv", name=f"br{bi}_{c}")
                    nc.gpsimd.dma_start(
                        out=brow[:],
                        in_=bass.AP(tensor=bias_src, offset=c * 512,
                                    ap=[[0, 1], [1, 512]]),
                    )
                    pb = psum.tile([P, 512], f32, tag="ps", name=f"pb{bi}_{c}")
                    nc.tensor.matmul(
                        pb[:], ones_row[:, :], brow[:], start=True, stop=True
                    )
                    nc.vector.tensor_copy(bias_dst[:, c * 512:(c + 1) * 512], pb[:])

            # ---- bulk DMAs in consumption order ----
            # q-phase A operands first so PE ramps immediately
            for i in range(NT):
                nc.sync.dma_start(out=wq8[:, i, :], in_=wq8_h[i * P:(i + 1) * P, :])
            for i in range(NT):
                nc.sync.dma_start(
                    out=xT8a[:, i, :], in_=xT8_h[i * P:(i + 1) * P, 0:b_core // 2]
                )
            # k-phase operands
            for c in range(2):
                for i in range(NT):
                    nc.sync.dma_start(
                        out=wk16[c][:, i, :],
                        in_=wk16_h[i * P:(i + 1) * P, c * 512:(c + 1) * 512],
                    )
            for i in range(NT):
                nc.sync.dma_start(
                    out=xT16[:, i, :], in_=xT16_h[i * P:(i + 1) * P, :]
                )
            # M-phase + pd operands
            for t in range(nbt):
                nc.sync.dma_start(
                    out=xn8[:, t, :], in_=xn8_h[t * P:(t + 1) * P, :]
                )
            for i in range(NT):
                nc.sync.dma_start(out=wv16[:, i, :], in_=wv16_h[i * P:(i + 1) * P, :])
            # q-phase B operand last
            for i in range(NT):
                nc.sync.dma_start(
                    out=xT8b[:, i, :], in_=xT8_h[i * P:(i + 1) * P, b_core // 2:]
                )

            def emit_q(xt8_half, half):
                """et8 = exp(qT + bq - SHIFT), transposed layout [h, b]. fp8 DR."""
                for hb in range(NT):
                    for bc in range(nbc // 2):
                        pq = psum.tile([P, 512], f32, tag="ps",
                                       name=f"pq{half}_{hb}_{bc}")
                        for j in range(NT // 2):
                            nc.tensor.matmul(
                                pq[:],
                                wq8[:, 2 * j:2 * j + 2, hb * P:(hb + 1) * P],
                                xt8_half[:, 2 * j:2 * j + 2, bc * 512:(bc + 1) * 512],
                                start=(j == 0), stop=(j == NT // 2 - 1),
                                perf_mode=DR,
                            )
                        off = half * (b_core // 2) + bc * 512
                        nc.scalar.activation(
                            et8[:, hb, off:off + 512], pq[:], AF.Exp,
                            bias=bq_c[:, hb:hb + 1], scale=q_act_scale,
                        )

            def emit_k():
                """g8 = lr * sigmoid(k) * SG, natural layout [b, h]. bf16."""
                for t in range(nbt):
                    ps = [
                        psum.tile([P, 512], f32, tag="ps", name=f"pk{t}_{c}")
                        for c in range(2)
                    ]
                    plr = psmall.tile([P, 1], f32, tag="pl", name=f"plr{t}")
                    for i in range(NT):
                        lhs = xT16[:, i, t * P:(t + 1) * P]
                        st = dict(start=(i == 0), stop=(i == NT - 1))
                        for c in range(2):
                            nc.tensor.matmul(ps[c][:], lhs, wk16[c][:, i, :], **st)
                        nc.tensor.matmul(plr[:], lhs, wlr16[:, i, :], **st)
                    # lr_sg = sigmoid(lr + blr) * SG
                    lr_s = small.tile([P, 1], f32, tag="lr", name=f"lr{t}")
                    nc.scalar.activation(lr_s[:], plr[:], AF.Sigmoid, bias=blr_c[:, 0:1])
                    nc.vector.tensor_scalar_mul(lr_s[:], lr_s[:], SG)
                    for c in range(2):
                        ktmp = tmp.tile([P, 512], f32, tag="kv", name=f"kt{t}_{c}")
                        nc.vector.tensor_add(
                            ktmp[:], ps[c][:], bk_b[:, c * 512:(c + 1) * 512]
                        )
                        sgk = tmp.tile([P, 512], bf16, tag="sg", name=f"sg{t}_{c}")
                        nc.scalar.activation(sgk[:], ktmp[:], AF.Sigmoid)
                        nc.scalar.activation(
                            g8[:, t, c * 512:(c + 1) * 512], sgk[:], AF.Copy,
                            scale=lr_s[:],
                        )

            def emit_m():
                """mb = x.T @ g (per-core partial), [i, h] layout, fp8 DR."""
                for hc in range(2):
                    for ib in range(NT):
                        pm = psum.tile([P, 512], f32, tag="ps", name=f"pm{hc}_{ib}")
                        for bp in range(nbp):
                            nc.tensor.matmul(
                                pm[:],
                                xn8[:, 2 * bp:2 * bp + 2, ib * P:(ib + 1) * P],
                                g8[:, 2 * bp:2 * bp + 2, hc * 512:(hc + 1) * 512],
                                start=(bp == 0), stop=(bp == nbp - 1),
                                perf_mode=DR,
                            )
                        nc.vector.tensor_copy(
                            mb[:, ib, hc * 512:(hc + 1) * 512], pm[:]
                        )
                # r_c[p, hb] = sum_b g[b, hb*128+p]  (ones-rhs matmuls, [h, 1])
                for hb in range(NT):
                    pr = psmall.tile([P, 1], f32, tag="pl", name=f"pr{hb}")
                    for bt in range(nbt):
                        nc.tensor.matmul(
                            pr[:],
                            g8[:, bt, hb * P:(hb + 1) * P],
                            ones8[:, 0, 0:1],
                            start=(bt == 0), stop=(bt == nbt - 1),
                        )
                    nc.vector.tensor_copy(r_c[:, hb:hb + 1], pr[:])

            def emit_pd():
                """delta.T partial = mb.T @ wv + r x bvc, drained fp8 to ar_in."""
                for hb in range(NT):
                    pd = [
                        psum.tile([P, 512], f32, tag="ps", name=f"pd{hb}_{oc}")
                        for oc in range(2)
                    ]
                    for oc in range(2):
                        for i in range(NT):
                            nc.tensor.matmul(
                                pd[oc][:],
                                mb[:, i, hb * P:(hb + 1) * P],
                                wv16[:, i, oc * 512:(oc + 1) * 512],
                                start=(i == 0), stop=(i == NT - 1),
                            )
                    dst = arst.tile([P, DIM], bf16, tag="ar", name=f"ds{hb}")
                    for oc in range(2):
                        # psum*pds -> tmp, then + r[h] * bvcp[o] on DVE
                        pt = tmp.tile([P, 512], f32, tag="kv", name=f"pt{hb}_{oc}")
                        nc.scalar.activation(
                            pt[:], pd[oc][:], AF.Copy, scale=pd_drain_scale
                        )
                        nc.vector.scalar_tensor_tensor(
                            dst[:, oc * 512:(oc + 1) * 512],
                            bvcp_b[:, oc * 512:(oc + 1) * 512],
                            r_c[:, hb:hb + 1],
                            pt[:],
                            op0=ALU.mult,
                            op1=ALU.add,
                        )
                    nc.sync.dma_start(
                        out=ar_in[hb * P:(hb + 1) * P, :], in_=dst[:]
                    )

            def emit_fin():
                """out = (et8.T @ wn8) / rowsum + wfb, fp8 DR."""
                for t in range(nbt):
                    po = [
                        psum.tile([P, 512], f32, tag="ps", name=f"po{t}_{oc}")
                        for oc in range(2)
                    ]
                    prs = psmall.tile([P, 1], f32, tag="pl", name=f"prs{t}")
                    for j in range(NT // 2):
                        lhs = et8[:, 2 * j:2 * j + 2, t * P:(t + 1) * P]
                        st = dict(start=(j == 0), stop=(j == NT // 2 - 1))
                        for oc in range(2):
                            nc.tensor.matmul(
                                po[oc][:], lhs,
                                wn8[:, 2 * j:2 * j + 2, oc * 512:(oc + 1) * 512],
                                perf_mode=DR, **st,
                            )
                    for hb in range(NT):
                        nc.tensor.matmul(
                            prs[:], et8[:, hb, t * P:(t + 1) * P], ones8[:, 0, 0:1],
                            start=(hb == 0), stop=(hb == NT - 1),
                        )
                    recip = small.tile([P, 1], f32, tag="rc", name=f"rc{t}")
                    nc.vector.reciprocal(recip[:], prs[:])
                    nc.vector.tensor_scalar_mul(recip[:], recip[:], fin_recip_scale)
                    o_st = ost.tile([P, DIM], f32, tag="os", name=f"os{t}")
                    for oc in range(2):
                        nc.vector.scalar_tensor_tensor(
                            o_st[:, oc * 512:(oc + 1) * 512],
                            po[oc][:],
                            recip[:],
                            wfb_b[:, oc * 512:(oc + 1) * 512],
                            op0=ALU.mult,
                            op1=ALU.add,
                        )
                    nc.sync.dma_start(out=out_h[t * P:(t + 1) * P, :], in_=o_st[:])

            # ---- schedule ----
            emit_q(xT8a, 0)       # covers k-operand DMA; ramps PE early
            emit_k()
            emit_m()
            emit_pd()
            nc.gpsimd.collective_compute(
                "AllReduce",
                mybir.AluOpType.add,
                replica_groups=[list(range(n_cores))],
                ins=[ar_in[:, :]],
                outs=[ar_out[:, :]],
            )
            emit_q(xT8b, 1)       # hides the AllReduce
            for hb in range(NT):
                wst = arst.tile([P, DIM], bf16, tag="ar", name=f"wst{hb}")
                nc.sync.dma_start(
                    out=wst[:], in_=ar_out[hb * P:(hb + 1) * P, :]
                )
                nc.scalar.activation(wn8[:, hb, :], wst[:], AF.Copy)
            emit_fin()

    nc.compile()
    return nc


def _host_prep(x, W_slow_w, W_slow_b, W_fast_b, b_core, n_cores):
    """Shard + pre-transpose + cast inputs; returns per-core input maps."""
    Wk = W_slow_w[:DIM]
    Wv = W_slow_w[DIM:2 * DIM]
    Wq = W_slow_w[2 * DIM:3 * DIM]
    wlr = W_slow_w[3 * DIM]

    wk16 = np.ascontiguousarray(Wk.T).astype(BF16)
    wv16 = np.ascontiguousarray(Wv.T).astype(BF16)
    wq8 = np.clip(np.ascontiguousarray(Wq.T) * SWQ, -240.0, 240.0).astype(F8E4)
    wlr16 = np.ascontiguousarray(wlr).astype(BF16)

    bk = np.ascontiguousarray(W_slow_b[:DIM]).astype(np.float32)
    # outer-product operand, pre-scaled so (bvcp * r_c) matches the drained
    # pd units: r_c = r_true*SG, want r_true*bvc*SAR/B -> bvcp = bvc*SAR/(B*SG)
    b_total = float(B_FULL)
    bvcp = ((W_slow_b[DIM:2 * DIM] - W_fast_b) * (SAR / (b_total * SG))).astype(
        np.float32
    )
    bqs = (W_slow_b[2 * DIM:3 * DIM] - SHIFT).astype(np.float32)
    blr = np.ascontiguousarray(W_slow_b[3 * DIM:3 * DIM + 1]).astype(np.float32)
    wfb = np.ascontiguousarray(W_fast_b).astype(np.float32)

    in_maps = []
    for c in range(n_cores):
        xs = x[c * b_core:(c + 1) * b_core, :]
        xT = np.ascontiguousarray(xs.T)
        xT16 = xT.astype(BF16)
        xT8 = np.clip(xT * SX, -240.0, 240.0).astype(F8E4)
        xn8 = np.clip(xs * SX, -240.0, 240.0).astype(F8E4)
        in_maps.append({
            "xT16": xT16, "xT8": xT8, "xn8": np.ascontiguousarray(xn8),
            "wk16": wk16, "wq8": wq8, "wv16": wv16, "wlr16": wlr16,
            "bk": bk, "bqs": bqs, "blr": blr, "bvcp": bvcp, "wfb": wfb,
        })
    return in_maps


_PROGRAM_CACHE = {}


def _get_program(b_core, n_cores=N_CORES):
    key = (b_core, n_cores)
    if key not in _PROGRAM_CACHE:
        _PROGRAM_CACHE[key] = _build_program(b_core, n_cores)
    return _PROGRAM_CACHE[key]


def _run_device(x, W_slow_w, W_slow_b, W_fast_b, trace=False):
    from concourse.bass_utils import run_bass_kernel_spmd

    b_core = x.shape[0] // N_CORES
    nc = _get_program(b_core)
    in_maps = _host_prep(x, W_slow_w, W_slow_b, W_fast_b, b_core, N_CORES)
    res = run_bass_kernel_spmd(nc, in_maps, list(range(N_CORES)), trace=trace)
    out = np.concatenate([res.results[c]["out"] for c in range(N_CORES)], axis=0)
    return out.astype(np.float32), res


def _reference_numpy(x, W_slow_w, W_slow_b, W_fast_w, W_fast_b):
    """Exact fallback (only used if W_fast_w != 0, which the spec never produces)."""
    x = x.astype(np.float64)
    s = x @ W_slow_w.astype(np.float64).T + W_slow_b.astype(np.float64)
    k = s[:, :DIM]
    v = s[:, DIM:2 * DIM]
    q = s[:, 2 * DIM:3 * DIM]
    lr = 1.0 / (1.0 + np.exp(-s[:, -1:]))
    ek = np.exp(k - k.max(axis=1, keepdims=True))
    ak = ek / ek.sum(axis=1, keepdims=True)
    v_bar = ak @ W_fast_w.astype(np.float64).T + W_fast_b.astype(np.float64)
    sigk = 1.0 / (1.0 + np.exp(-k))
    delta = (lr * (v - v_bar)).T @ sigk / x.shape[0]
    w_new = W_fast_w.astype(np.float64) + delta
    eq = np.exp(q - q.max(axis=1, keepdims=True))
    aq = eq / eq.sum(axis=1, keepdims=True)
    return (aq @ w_new.T + W_fast_b.astype(np.float64)).astype(np.float32)


def kernel(x, W_slow_w, W_slow_b, W_fast_w, W_fast_b):
    x = np.asarray(x)
    W_slow_w = np.asarray(W_slow_w)
    W_slow_b = np.asarray(W_slow_b)
    W_fast_w = np.asarray(W_fast_w)
    W_fast_b = np.asarray(W_fast_b)
    if np.any(W_fast_w):
        # Spec guarantees W_fast_w == 0; exact fallback for generality.
        return _reference_numpy(x, W_slow_w, W_slow_b, W_fast_w, W_fast_b)
    out, _ = _run_device(x, W_slow_w, W_slow_b, W_fast_b, trace=False)
    return out


# revision 13
# speedup vs baseline: 1.1834x; 1.0765x over previous
"""DeltaNet fused kernel for 8 TRN2 NeuronCores (Bass/Tile), fp8-hybrid v2.

Math (reference, with W_fast_w == 0 so v_bar == W_fast_b):
    s  = x @ W_slow_w.T + W_slow_b            [B, 3073]
    k  = s[:, :1024]; v = s[:, 1024:2048]; q = s[:, 2048:3072]
    lr = sigmoid(s[:, 3072])
    delta[o,h] = sum_b (lr*(v - wfb))[b,o] * sigmoid(k)[b,h] / B
    out = softmax(q) @ delta.T + wfb

Restructured to eliminate the v projection (v = x @ Wv.T + bv):
    g  = lr * sigmoid(k)                      [B, H]
    M  = x.T @ g                              [I, H]   (per-core partial)
    r  = sum_b g[b, :]                        [H]
    delta.T = (M.T @ Wv.T + r x (bv - wfb)) / B        [H, O]  (AllReduced)
    out = softmax(q) @ delta.T + wfb

Precision (validated in numpy simulation; end-to-end rel err 6.6e-3 vs 2e-2
tolerance): fp8-e4m3 DoubleRow for q / M / final matmuls and the first half
of the k contraction; bf16 for the rest of k, lr, and the delta.T matmul.
The AllReduce runs in fp8 (delta scaled by 4096), split into two [H, 512]
column-halves so the final matmul's first half can start after the first AR.

Schedule: q chunk 0 warms up the PE while the k operands stream in; the
whole rest of the q-phase plus the softmax row-sums run after the AR
trigger to hide the collective; the final matmul is split into per-AR-half
passes. lr / r / rowsum are computed as transposed [1, N] matmuls (cheap
N=512 streams instead of 384 N=1 matmuls) and moved cross-partition via
tiny DRAM round-trips.
"""

import os
import sys

for _p in ("/opt/trn_rl_repo", "/root/.axon_site/_ro/trn_rl_repo"):
    if os.path.isdir(_p) and _p not in sys.path:
        sys.path.append(_p)

import numpy as np
import ml_dtypes

BF16 = ml_dtypes.bfloat16
F8E4 = ml_dtypes.float8_e4m3     # TRN fp8e4: max normal +-240

N_CORES = 8
B_FULL = 16384
DIM = 1024          # dim_in == dim_out == dim_hidden
P = 128
NT = DIM // P       # 8 tiles along any 1024 dim

SX = 16.0           # x fp8 scale             (|x|max 5.4  -> 87)
SWQ = 512.0         # Wq fp8 scale            (|w|max .16  -> 80)
SWK = 512.0         # Wk fp8 scale (i < 512 half)
SKP = SX * SWK      # k psum scale (bf16 half pre-scaled to match)
SG = 128.0          # g fp8 scale             (g in (0,1)  -> <128)
SAR = 4096.0        # delta fp8 scale         (|delta|max .043 -> 176)
SHIFT = 3.0         # exp shift               (max q+bq 7.63 -> et < 103)


def _build_program(b_core: int, n_cores: int = N_CORES):
    """Build the SPMD Bass program (same program on every core)."""
    import concourse.bass as bass
    import concourse.mybir as mybir
    import concourse.tile as tile
    from concourse import bacc

    f32 = mybir.dt.float32
    bf16 = mybir.dt.bfloat16
    f8 = mybir.dt.float8e4
    AF = mybir.ActivationFunctionType
    ALU = mybir.AluOpType
    DR = mybir.MatmulPerfMode.DoubleRow

    nbt = b_core // P               # b-tiles per core (16)
    nbc = b_core // 512             # 512-wide b-chunks (4)
    nct = 512 // P                  # b-tiles per chunk (4)
    assert b_core % 1024 == 0

    nc = bacc.Bacc(
        "TRN2",
        target_bir_lowering=False,
        debug=False,
        num_devices=n_cores,
    )

    # ---- kernel I/O ----
    xT16_h = nc.dram_tensor("xT16", [DIM, b_core], bf16, kind="ExternalInput")
    xT8_h = nc.dram_tensor("xT8", [DIM, b_core], f8, kind="ExternalInput")
    xn8_h = nc.dram_tensor("xn8", [b_core, DIM], f8, kind="ExternalInput")
    wk8_h = nc.dram_tensor("wk8", [512, DIM], f8, kind="ExternalInput")
    wk16_h = nc.dram_tensor("wk16", [512, DIM], bf16, kind="ExternalInput")
    wq8_h = nc.dram_tensor("wq8", [DIM, DIM], f8, kind="ExternalInput")
    wv16_h = nc.dram_tensor("wv16", [DIM, DIM], bf16, kind="ExternalInput")
    wlr16_h = nc.dram_tensor("wlr16", [DIM], bf16, kind="ExternalInput")
    bk_h = nc.dram_tensor("bk", [DIM], f32, kind="ExternalInput")     # bk * SKP
    bqs_h = nc.dram_tensor("bqs", [DIM], f32, kind="ExternalInput")   # bq - SHIFT
    blr_h = nc.dram_tensor("blr", [1], f32, kind="ExternalInput")
    # (bv - wfb) * SAR / (b_total * SG): outer-product operand for the drain
    bvcp_h = nc.dram_tensor("bvcp", [DIM], f32, kind="ExternalInput")
    wfb_h = nc.dram_tensor("wfb", [DIM], f32, kind="ExternalInput")
    out_h = nc.dram_tensor("out", [b_core, DIM], bf16, kind="ExternalOutput")

    pd_drain_scale = SAR / (float(b_core * n_cores) * SX * SG)
    fin_recip_scale = 1.0 / SAR
    q_act_scale = 1.0 / (SX * SWQ)
    k_act_scale = 1.0 / SKP

    with tile.TileContext(nc) as tc:
        with (
            tc.tile_pool(name="persist", bufs=1) as persist,
            tc.tile_pool(name="psum", bufs=6, space="PSUM") as psum,
            tc.tile_pool(name="psmall", bufs=2, space="PSUM") as psmall,
            tc.tile_pool(name="tmp", bufs=4) as tmp,
            tc.tile_pool(name="ost", bufs=4) as ost,
            tc.tile_pool(name="arst", bufs=4) as arst,
            tc.tile_pool(name="dram", bufs=1, space="DRAM") as dram,
        ):
            # ---- persistent SBUF tensors ----
            wq8 = persist.tile([P, NT, DIM], f8, name="wq8")
            xT8a = persist.tile([P, NT, b_core // 2], f8, name="xT8a")
            xT8b = persist.tile([P, NT, b_core // 2], f8, name="xT8b")
            wk8 = persist.tile([P, 4, DIM], f8, name="wk8")
            wk16 = persist.tile([P, 4, DIM], bf16, name="wk16")
            wlr16 = persist.tile([P, NT, 1], bf16, name="wlr16")
            xT16 = persist.tile([P, NT, b_core], bf16, name="xT16")
            xn8 = persist.tile([P, nbt, DIM], f8, name="xn8")
            wv16 = persist.tile([P, NT, DIM], bf16, name="wv16")
            g8 = persist.tile([P, nbt, DIM], f8, name="g8")
            et8 = persist.tile([P, NT, b_core], f8, name="et8")
            mb = persist.tile([P, NT, DIM], bf16, name="mb")
            wn8a = persist.tile([P, NT, 512], f8, name="wn8a")
            wn8b = persist.tile([P, NT, 512], f8, name="wn8b")
            bk_b = persist.tile([P, DIM], f32, name="bk_b")
            wfb_b = persist.tile([P, DIM], f32, name="wfb_b")
            bvcp_b = persist.tile([P, DIM], f32, name="bvcp_b")
            bq_c = persist.tile([P, NT], f32, name="bq_c")
            blr_c = persist.tile([P, 1], f32, name="blr_c")
            lr_c = persist.tile([P, nbt], f32, name="lr_c")
            r_c = persist.tile([P, NT], f32, name="r_c")
            recip_c = persist.tile([P, nbt], f32, name="recip_c")
            lrT_sb = persist.tile([1, b_core], f32, name="lrT_sb")
            rT_sb = persist.tile([1, DIM], f32, name="rT_sb")
            prsT_sb = persist.tile([1, b_core], f32, name="prsT_sb")
            ones8 = persist.tile([P, 2, 16], f8, name="ones8")
            ones_row = persist.tile([1, P], f32, name="ones_row")

            # ---- DRAM: AllReduce bounce (column halves) + transpose scratch ----
            ar_inA = dram.tile([DIM, 512], f8, name="ar_inA")
            ar_inB = dram.tile([DIM, 512], f8, name="ar_inB")
            ar_outA = dram.tile([DIM, 512], f8, name="ar_outA", addr_space="Shared")
            ar_outB = dram.tile([DIM, 512], f8, name="ar_outB", addr_space="Shared")
            sc_lr = dram.tile([nbt, P], f32, name="sc_lr")
            sc_r = dram.tile([NT, P], f32, name="sc_r")
            sc_prs = dram.tile([nbt, P], f32, name="sc_prs")

            nc.vector.memset(ones8[:], 1.0)
            nc.vector.memset(ones_row[:], 1.0)

            # ---- small DMAs (gpsimd queue) ----
            nc.gpsimd.dma_start(
                out=bq_c[:],
                in_=bass.AP(tensor=bqs_h, offset=0, ap=[[1, P], [P, NT]]),
            )
            nc.gpsimd.dma_start(
                out=blr_c[:],
                in_=bass.AP(tensor=blr_h, offset=0, ap=[[0, P], [1, 1]]),
            )
            for i in range(NT):
                nc.gpsimd.dma_start(
                    out=wlr16[:, i, :],
                    in_=bass.AP(tensor=wlr16_h, offset=i * P, ap=[[1, P], [P, 1]]),
                )
            # bias broadcasts across partitions via K=1 ones-matmuls
            for bi, (bias_dst, bias_src) in enumerate(
                ((bk_b, bk_h), (wfb_b, wfb_h), (bvcp_b, bvcp_h))
            ):
                for c in range(2):
                    brow = tmp.tile([1, 512], f32, tag="kv", name=f"br{bi}_{c}")
                    nc.gpsimd.dma_start(
                        out=brow[:],
                        in_=bass.AP(tensor=bias_src, offset=c * 512,
                                    ap=[[0, 1], [1, 512]]),
                    )
                    pb = psum.tile([P, 512], f32, tag="ps", name=f"pb{bi}_{c}")
                    nc.tensor.matmul(
                        pb[:], ones_row[:, :], brow[:], start=True, stop=True
                    )
                    nc.vector.tensor_copy(bias_dst[:, c * 512:(c + 1) * 512], pb[:])

            # ---- bulk DMAs: sync queue = q/k weights + xT8; scalar = rest ----
            for i in range(NT):
                nc.sync.dma_start(out=wq8[:, i, :], in_=wq8_h[i * P:(i + 1) * P, :])
            for i in range(NT):
                nc.sync.dma_start(
                    out=xT8a[:, i, :], in_=xT8_h[i * P:(i + 1) * P, 0:b_core // 2]
                )
            for i in range(4):
                nc.sync.dma_start(out=wk8[:, i, :], in_=wk8_h[i * P:(i + 1) * P, :])
            for i in range(4):
                nc.sync.dma_start(out=wk16[:, i, :], in_=wk16_h[i * P:(i + 1) * P, :])
            for i in range(NT):
                nc.sync.dma_start(
                    out=xT8b[:, i, :], in_=xT8_h[i * P:(i + 1) * P, b_core // 2:]
                )
            # scalar queue: xT16 b-chunked (earliest b first), then M operands
            for c in range(nbc):
                for i in range(NT):
                    nc.scalar.dma_start(
                        out=xT16[:, i, c * 512:(c + 1) * 512],
                        in_=xT16_h[i * P:(i + 1) * P, c * 512:(c + 1) * 512],
                    )
            for t in range(nbt):
                nc.scalar.dma_start(out=xn8[:, t, :], in_=xn8_h[t * P:(t + 1) * P, :])
            for i in range(NT):
                nc.scalar.dma_start(out=wv16[:, i, :], in_=wv16_h[i * P:(i + 1) * P, :])

            def xt8_lhs(t, j2):
                """fp8 xT lhsT pair slice for global b-tile t, i-pair j2."""
                src = xT8a if t < nbt // 2 else xT8b
                tc_ = t % (nbt // 2)
                return src[:, 2 * j2:2 * j2 + 2, tc_ * P:(tc_ + 1) * P]

            def emit_q(chunks):
                """et8 = exp(qT + bq - SHIFT), transposed layout [h, b]. fp8 DR."""
                for bc in chunks:
                    src = xT8a if bc < nbc // 2 else xT8b
                    lo = (bc % (nbc // 2)) * 512
                    for hb in range(NT):
                        pq = psum.tile([P, 512], f32, tag="ps", name=f"pq{bc}_{hb}")
                        for j in range(NT // 2):
                            nc.tensor.matmul(
                                pq[:],
                                wq8[:, 2 * j:2 * j + 2, hb * P:(hb + 1) * P],
                                src[:, 2 * j:2 * j + 2, lo:lo + 512],
                                start=(j == 0), stop=(j == NT // 2 - 1),
                                perf_mode=DR,
                            )
                        nc.scalar.activation(
                            et8[:, hb, bc * 512:(bc + 1) * 512], pq[:], AF.Exp,
                            bias=bq_c[:, hb:hb + 1], scale=q_act_scale,
                        )

            def emit_lrT():
                """lr_c[p, t] = sigmoid(x @ wlr + blr)[t*128+p] * SG via
                transposed [1, b] matmuls + DRAM bounce."""
                for bc in range(nbc):
                    pl = psmall.tile([1, 512], f32, tag="pl", name=f"plr{bc}")
                    for i in range(NT):
                        nc.tensor.matmul(
                            pl[:],
                            wlr16[:, i, 0:1],
                            xT16[:, i, bc * 512:(bc + 1) * 512],
                            start=(i == 0), stop=(i == NT - 1),
                        )
                    nc.scalar.activation(
                        lrT_sb[0:1, bc * 512:(bc + 1) * 512], pl[:], AF.Sigmoid,
                        bias=blr_c[0:1, 0:1],
                    )
                    nc.gpsimd.dma_start(
                        out=sc_lr[nct * bc:nct * (bc + 1), :],
                        in_=lrT_sb[0:1, bc * 512:(bc + 1) * 512],
                    )
                nc.gpsimd.dma_start(
                    out=lr_c[:, :], in_=sc_lr[:, :].rearrange("a b -> b a")
                )
                nc.vector.tensor_scalar_mul(lr_c[:], lr_c[:], SG)

            def emit_k():
                """g8 = lr * sigmoid(k) * SG, natural layout [b, h].
                Contraction split: i<512 fp8-DR, i>=512 bf16."""
                for t in range(nbt):
                    for c in range(2):
                        pk = psum.tile([P, 512], f32, tag="ps", name=f"pk{t}_{c}")
                        for j2 in range(2):
                            nc.tensor.matmul(
                                pk[:],
                                xt8_lhs(t, j2),
                                wk8[:, 2 * j2:2 * j2 + 2, c * 512:(c + 1) * 512],
                                start=(j2 == 0), stop=False,
                                perf_mode=DR,
                            )
                        for i in range(4):
                            nc.tensor.matmul(
                                pk[:],
                                xT16[:, 4 + i, t * P:(t + 1) * P],
                                wk16[:, i, c * 512:(c + 1) * 512],
                                start=False, stop=(i == 3),
                            )
                        ktmp = tmp.tile([P, 512], f32, tag="kv", name=f"kt{t}_{c}")
                        nc.vector.tensor_add(
                            ktmp[:], pk[:], bk_b[:, c * 512:(c + 1) * 512]
                        )
                        sgk = tmp.tile([P, 512], bf16, tag="sg", name=f"sg{t}_{c}")
                        nc.scalar.activation(sgk[:], ktmp[:], AF.Sigmoid,
                                             scale=k_act_scale)
                        nc.scalar.activation(
                            g8[:, t, c * 512:(c + 1) * 512], sgk[:], AF.Copy,
                            scale=lr_c[:, t:t + 1],
                        )

            def emit_m():
                """mb = x.T @ g (per-core partial), [i, h] layout, fp8 DR;
                then rT = ones.T @ g via [1, 512] matmuls + bounce."""
                for hc in range(2):
                    for ib in range(NT):
                        pm = psum.tile([P, 512], f32, tag="ps", name=f"pm{hc}_{ib}")
                        for bp in range(nbt // 2):
                            nc.tensor.matmul(
                                pm[:],
                                xn8[:, 2 * bp:2 * bp + 2, ib * P:(ib + 1) * P],
                                g8[:, 2 * bp:2 * bp + 2, hc * 512:(hc + 1) * 512],
                                start=(bp == 0), stop=(bp == nbt // 2 - 1),
                                perf_mode=DR,
                            )
                        nc.vector.tensor_copy(
                            mb[:, ib, hc * 512:(hc + 1) * 512], pm[:]
                        )
                for hc in range(2):
                    pr = psmall.tile([1, 512], f32, tag="pl", name=f"pr{hc}")
                    for bt in range(nbt):
                        nc.tensor.matmul(
                            pr[:],
                            ones8[:, 0, 0:1],
                            g8[:, bt, hc * 512:(hc + 1) * 512],
                            start=(bt == 0), stop=(bt == nbt - 1),
                        )
                    nc.vector.tensor_copy(
                        rT_sb[0:1, hc * 512:(hc + 1) * 512], pr[:]
                    )
                    nc.gpsimd.dma_start(
                        out=sc_r[nct * hc:nct * (hc + 1), :],
                        in_=rT_sb[0:1, hc * 512:(hc + 1) * 512],
                    )
                nc.gpsimd.dma_start(
                    out=r_c[:, :], in_=sc_r[:, :].rearrange("a b -> b a")
                )

            def emit_pd():
                """delta.T partial = mb.T @ wv + r x bvc, drained fp8 to the
                two AR column-halves."""
                for hb in range(NT):
                    for oc in range(2):
                        pd = psum.tile([P, 512], f32, tag="ps", name=f"pd{hb}_{oc}")
                        for i in range(NT):
                            nc.tensor.matmul(
                                pd[:],
                                mb[:, i, hb * P:(hb + 1) * P],
                                wv16[:, i, oc * 512:(oc + 1) * 512],
                                start=(i == 0), stop=(i == NT - 1),
                            )
                        pt = tmp.tile([P, 512], f32, tag="kv", name=f"pt{hb}_{oc}")
                        nc.scalar.activation(
                            pt[:], pd[:], AF.Copy, scale=pd_drain_scale
                        )
                        dst = arst.tile([P, 512], f8, tag="ar", name=f"ds{hb}_{oc}")
                        nc.vector.scalar_tensor_tensor(
                            dst[:],
                            bvcp_b[:, oc * 512:(oc + 1) * 512],
                            r_c[:, hb:hb + 1],
                            pt[:],
                            op0=ALU.mult,
                            op1=ALU.add,
                        )
                        ar_dst = ar_inA if oc == 0 else ar_inB
                        eng = nc.sync if oc == 0 else nc.scalar
                        eng.dma_start(
                            out=ar_dst[hb * P:(hb + 1) * P, :], in_=dst[:]
                        )

            def emit_prsT(bc):
                """prsT[b] = sum_h et8[h, b] for one 512-col chunk."""
                pp = psmall.tile([1, 512], f32, tag="pl", name=f"pp{bc}")
                for hb in range(NT):
                    nc.tensor.matmul(
                        pp[:],
                        ones8[:, 0, 0:1],
                        et8[:, hb, bc * 512:(bc + 1) * 512],
                        start=(hb == 0), stop=(hb == NT - 1),
                    )
                nc.vector.tensor_copy(prsT_sb[0:1, bc * 512:(bc + 1) * 512], pp[:])
                nc.gpsimd.dma_start(
                    out=sc_prs[nct * bc:nct * (bc + 1), :],
                    in_=prsT_sb[0:1, bc * 512:(bc + 1) * 512],
                )

            def emit_recip():
                nc.gpsimd.dma_start(
                    out=recip_c[:, :], in_=sc_prs[:, :].rearrange("a b -> b a")
                )
                nc.vector.reciprocal(recip_c[:], recip_c[:])
                nc.vector.tensor_scalar_mul(recip_c[:], recip_c[:], fin_recip_scale)

            def emit_fin(oc, wn):
                """out[:, oc half] = (et8.T @ wn) * recip + wfb, fp8 DR."""
                for t in range(nbt):
                    po = psum.tile([P, 512], f32, tag="ps", name=f"po{t}_{oc}")
                    for j in range(NT // 2):
                        nc.tensor.matmul(
                            po[:],
                            et8[:, 2 * j:2 * j + 2, t * P:(t + 1) * P],
                            wn[:, 2 * j:2 * j + 2, :],
                            start=(j == 0), stop=(j == NT // 2 - 1),
                            perf_mode=DR,
                        )
                    o_st = ost.tile([P, 512], bf16, tag="os", name=f"os{t}_{oc}")
                    nc.vector.scalar_tensor_tensor(
                        o_st[:],
                        po[:],
                        recip_c[:, t:t + 1],
                        wfb_b[:, oc * 512:(oc + 1) * 512],
                        op0=ALU.mult,
                        op1=ALU.add,
                    )
                    eng = nc.sync if oc == 0 else nc.scalar
                    eng.dma_start(
                        out=out_h[t * P:(t + 1) * P, oc * 512:(oc + 1) * 512],
                        in_=o_st[:],
                    )

            # ---- schedule ----
            emit_q([0])           # warm up PE while k operands stream in
            emit_lrT()
            emit_k()
            emit_m()
            emit_pd()
            for half, (ai, ao) in enumerate(((ar_inA, ar_outA), (ar_inB, ar_outB))):
                nc.gpsimd.collective_compute(
                    "AllReduce",
                    mybir.AluOpType.add,
                    replica_groups=[list(range(n_cores))],
                    ins=[ai[:, :]],
                    outs=[ao[:, :]],
                )
            emit_q(list(range(1, nbc)))     # hides the AllReduce
            for bc in range(nbc):
                emit_prsT(bc)
            emit_recip()
            for hb in range(NT):
                nc.sync.dma_start(
                    out=wn8a[:, hb, :], in_=ar_outA[hb * P:(hb + 1) * P, :]
                )
            emit_fin(0, wn8a)
            for hb in range(NT):
                nc.scalar.dma_start(
                    out=wn8b[:, hb, :], in_=ar_outB[hb * P:(hb + 1) * P, :]
                )
            emit_fin(1, wn8b)

    nc.compile()
    return nc


def _host_prep(x, W_slow_w, W_slow_b, W_fast_b, b_core, n_cores):
    """Shard + pre-transpose + cast inputs; returns per-core input maps."""
    Wk = W_slow_w[:DIM]
    Wv = W_slow_w[DIM:2 * DIM]
    Wq = W_slow_w[2 * DIM:3 * DIM]
    wlr = W_slow_w[3 * DIM]

    WkT = np.ascontiguousarray(Wk.T)
    wk8 = np.clip(WkT[:512, :] * SWK, -240.0, 240.0).astype(F8E4)
    wk16 = (WkT[512:, :] * SKP).astype(BF16)
    wv16 = np.ascontiguousarray(Wv.T).astype(BF16)
    wq8 = np.clip(np.ascontiguousarray(Wq.T) * SWQ, -240.0, 240.0).astype(F8E4)
    wlr16 = np.ascontiguousarray(wlr).astype(BF16)

    bk = (W_slow_b[:DIM] * SKP).astype(np.float32)
    b_total = float(b_core * n_cores)
    bvcp = ((W_slow_b[DIM:2 * DIM] - W_fast_b) * (SAR / (b_total * SG))).astype(
        np.float32
    )
    bqs = (W_slow_b[2 * DIM:3 * DIM] - SHIFT).astype(np.float32)
    blr = np.ascontiguousarray(W_slow_b[3 * DIM:3 * DIM + 1]).astype(np.float32)
    wfb = np.ascontiguousarray(W_fast_b).astype(np.float32)

    in_maps = []
    for c in range(n_cores):
        xs = x[c * b_core:(c + 1) * b_core, :]
        xT = np.ascontiguousarray(xs.T)
        xT16 = xT.astype(BF16)
        xT8 = np.clip(xT * SX, -240.0, 240.0).astype(F8E4)
        xn8 = np.clip(xs * SX, -240.0, 240.0).astype(F8E4)
        in_maps.append({
            "xT16": xT16, "xT8": xT8, "xn8": np.ascontiguousarray(xn8),
            "wk8": wk8, "wk16": wk16, "wq8": wq8, "wv16": wv16, "wlr16": wlr16,
            "bk": bk, "bqs": bqs, "blr": blr, "bvcp": bvcp, "wfb": wfb,
        })
    return in_maps


_PROGRAM_CACHE = {}


def _get_program(b_core, n_cores=N_CORES):
    key = (b_core, n_cores)
    if key not in _PROGRAM_CACHE:
        _PROGRAM_CACHE[key] = _build_program(b_core, n_cores)
    return _PROGRAM_CACHE[key]


def _run_device(x, W_slow_w, W_slow_b, W_fast_b, trace=False):
    from concourse.bass_utils import run_bass_kernel_spmd

    b_core = x.shape[0] // N_CORES
    nc = _get_program(b_core)
    in_maps = _host_prep(x, W_slow_w, W_slow_b, W_fast_b, b_core, N_CORES)
    res = run_bass_kernel_spmd(nc, in_maps, list(range(N_CORES)), trace=trace)
    out = np.concatenate([res.results[c]["out"] for c in range(N_CORES)], axis=0)
    return out.astype(np.float32), res


def _reference_numpy(x, W_slow_w, W_slow_b, W_fast_w, W_fast_b):
    """Exact fallback (only used if W_fast_w != 0, which the spec never produces)."""
    x = x.astype(np.float64)
    s = x @ W_slow_w.astype(np.float64).T + W_slow_b.astype(np.float64)
    k = s[:, :DIM]
    v = s[:, DIM:2 * DIM]
    q = s[:, 2 * DIM:3 * DIM]
    lr = 1.0 / (1.0 + np.exp(-s[:, -1:]))
    ek = np.exp(k - k.max(axis=1, keepdims=True))
    ak = ek / ek.sum(axis=1, keepdims=True)
    v_bar = ak @ W_fast_w.astype(np.float64).T + W_fast_b.astype(np.float64)
    sigk = 1.0 / (1.0 + np.exp(-k))
    delta = (lr * (v - v_bar)).T @ sigk / x.shape[0]
    w_new = W_fast_w.astype(np.float64) + delta
    eq = np.exp(q - q.max(axis=1, keepdims=True))
    aq = eq / eq.sum(axis=1, keepdims=True)
    return (aq @ w_new.T + W_fast_b.astype(np.float64)).astype(np.float32)


def kernel(x, W_slow_w, W_slow_b, W_fast_w, W_fast_b):
    x = np.asarray(x)
    W_slow_w = np.asarray(W_slow_w)
    W_slow_b = np.asarray(W_slow_b)
    W_fast_w = np.asarray(W_fast_w)
    W_fast_b = np.asarray(W_fast_b)
    if np.any(W_fast_w):
        # Spec guarantees W_fast_w == 0; exact fallback for generality.
        return _reference_numpy(x, W_slow_w, W_slow_b, W_fast_w, W_fast_b)
    out, _ = _run_device(x, W_slow_w, W_slow_b, W_fast_b, trace=False)
    return out


# revision 18
# speedup vs baseline: 1.2546x; 1.0602x over previous
"""DeltaNet fused kernel for 8 TRN2 NeuronCores (Bass/Tile), fp8-hybrid v2.

Math (reference, with W_fast_w == 0 so v_bar == W_fast_b):
    s  = x @ W_slow_w.T + W_slow_b            [B, 3073]
    k  = s[:, :1024]; v = s[:, 1024:2048]; q = s[:, 2048:3072]
    lr = sigmoid(s[:, 3072])
    delta[o,h] = sum_b (lr*(v - wfb))[b,o] * sigmoid(k)[b,h] / B
    out = softmax(q) @ delta.T + wfb

Restructured to eliminate the v projection (v = x @ Wv.T + bv):
    g  = lr * sigmoid(k)                      [B, H]
    M  = x.T @ g                              [I, H]   (per-core partial)
    r  = sum_b g[b, :]                        [H]
    delta.T = (M.T @ Wv.T + r x (bv - wfb)) / B        [H, O]  (AllReduced)
    out = softmax(q) @ delta.T + wfb

Precision (validated in numpy simulation; end-to-end rel err 6.6e-3 vs 2e-2
tolerance): fp8-e4m3 DoubleRow for q / M / final matmuls and the first half
of the k contraction; bf16 for the rest of k, lr, and the delta.T matmul.
The AllReduce runs in fp8 (delta scaled by 4096), split into two [H, 512]
column-halves so the final matmul's first half can start after the first AR.

Schedule: q chunk 0 warms up the PE while the k operands stream in; the
whole rest of the q-phase plus the softmax row-sums run after the AR
trigger to hide the collective; the final matmul is split into per-AR-half
passes. lr / r / rowsum are computed as transposed [1, N] matmuls (cheap
N=512 streams instead of 384 N=1 matmuls) and moved cross-partition via
tiny DRAM round-trips.
"""

import os
import sys

for _p in ("/opt/trn_rl_repo", "/root/.axon_site/_ro/trn_rl_repo"):
    if os.path.isdir(_p) and _p not in sys.path:
        sys.path.append(_p)

import numpy as np
import ml_dtypes

BF16 = ml_dtypes.bfloat16
F8E4 = ml_dtypes.float8_e4m3     # TRN fp8e4: max normal +-240

N_CORES = 8
B_FULL = 16384
DIM = 1024          # dim_in == dim_out == dim_hidden
P = 128
NT = DIM // P       # 8 tiles along any 1024 dim

SX = 16.0           # x fp8 scale             (|x|max 5.4  -> 87)
SWQ = 512.0         # Wq fp8 scale            (|w|max .16  -> 80)
SWK = 512.0         # Wk fp8 scale (i < 512 half)
SKP = SX * SWK      # k psum scale (bf16 half pre-scaled to match)
SG = 128.0          # g fp8 scale             (g in (0,1)  -> <128)
SAR = 4096.0        # delta fp8 scale         (|delta|max .043 -> 176)
SHIFT = 3.0         # exp shift               (max q+bq 7.63 -> et < 103)


def _build_program(b_core: int, n_cores: int = N_CORES):
    """Build the SPMD Bass program (same program on every core)."""
    import concourse.bass as bass
    import concourse.mybir as mybir
    import concourse.tile as tile
    from concourse import bacc

    f32 = mybir.dt.float32
    bf16 = mybir.dt.bfloat16
    f8 = mybir.dt.float8e4
    AF = mybir.ActivationFunctionType
    ALU = mybir.AluOpType
    DR = mybir.MatmulPerfMode.DoubleRow

    nbt = b_core // P               # b-tiles per core (16)
    nbc = b_core // 512             # 512-wide b-chunks (4)
    nct = 512 // P                  # b-tiles per chunk (4)
    assert b_core % 1024 == 0

    nc = bacc.Bacc(
        "TRN2",
        target_bir_lowering=False,
        debug=False,
        num_devices=n_cores,
    )

    # ---- kernel I/O ----
    xT16_h = nc.dram_tensor("xT16", [DIM, b_core], bf16, kind="ExternalInput")
    xT8_h = nc.dram_tensor("xT8", [DIM, b_core], f8, kind="ExternalInput")
    xn8_h = nc.dram_tensor("xn8", [b_core, DIM], f8, kind="ExternalInput")
    wk8_h = nc.dram_tensor("wk8", [512, DIM], f8, kind="ExternalInput")
    wk16_h = nc.dram_tensor("wk16", [512, DIM], bf16, kind="ExternalInput")
    wq8_h = nc.dram_tensor("wq8", [DIM, DIM], f8, kind="ExternalInput")
    wv16_h = nc.dram_tensor("wv16", [DIM, DIM], bf16, kind="ExternalInput")
    wlr16_h = nc.dram_tensor("wlr16", [DIM], bf16, kind="ExternalInput")
    bk_h = nc.dram_tensor("bk", [DIM], f32, kind="ExternalInput")     # bk * SKP
    bqs_h = nc.dram_tensor("bqs", [DIM], f32, kind="ExternalInput")   # bq - SHIFT
    blr_h = nc.dram_tensor("blr", [1], f32, kind="ExternalInput")
    # (bv - wfb) * SAR / (b_total * SG): outer-product operand for the drain
    bvcp_h = nc.dram_tensor("bvcp", [DIM], f32, kind="ExternalInput")
    wfb_h = nc.dram_tensor("wfb", [DIM], f32, kind="ExternalInput")
    out_h = nc.dram_tensor("out", [b_core, DIM], bf16, kind="ExternalOutput")

    pd_drain_scale = SAR / (float(b_core * n_cores) * SX * SG)
    fin_recip_scale = 1.0 / SAR
    q_act_scale = 1.0 / (SX * SWQ)
    k_act_scale = 1.0 / SKP

    with tile.TileContext(nc) as tc:
        with (
            tc.tile_pool(name="persist", bufs=1) as persist,
            tc.tile_pool(name="psum", bufs=6, space="PSUM") as psum,
            tc.tile_pool(name="psmall", bufs=2, space="PSUM") as psmall,
            tc.tile_pool(name="tmp", bufs=4) as tmp,
            tc.tile_pool(name="ost", bufs=4) as ost,
            tc.tile_pool(name="arst", bufs=4) as arst,
            tc.tile_pool(name="dram", bufs=1, space="DRAM") as dram,
        ):
            # ---- persistent SBUF tensors ----
            wq8 = persist.tile([P, NT, DIM], f8, name="wq8")
            xT8a = persist.tile([P, NT, b_core // 2], f8, name="xT8a")
            xT8b = persist.tile([P, NT, b_core // 2], f8, name="xT8b")
            wk8 = persist.tile([P, 4, DIM], f8, name="wk8")
            wk16 = persist.tile([P, 4, DIM], bf16, name="wk16")
            wlr16 = persist.tile([P, NT, 1], bf16, name="wlr16")
            xT16 = persist.tile([P, NT, b_core], bf16, name="xT16")
            xn8 = persist.tile([P, nbt, DIM], f8, name="xn8")
            wv16 = persist.tile([P, NT, DIM], bf16, name="wv16")
            g8 = persist.tile([P, nbt, DIM], f8, name="g8")
            et8 = persist.tile([P, NT, b_core], f8, name="et8")
            mb = persist.tile([P, NT, DIM], bf16, name="mb")
            wn8a = persist.tile([P, NT, 512], f8, name="wn8a")
            wn8b = persist.tile([P, NT, 512], f8, name="wn8b")
            bk_b = persist.tile([P, DIM], f32, name="bk_b")
            wfb_b = persist.tile([P, DIM], f32, name="wfb_b")
            bvcp_b = persist.tile([P, DIM], f32, name="bvcp_b")
            bq_c = persist.tile([P, NT], f32, name="bq_c")
            blr_c = persist.tile([P, 1], f32, name="blr_c")
            lr_c = persist.tile([P, nbt], f32, name="lr_c")
            r_c = persist.tile([P, NT], f32, name="r_c")
            recip_c = persist.tile([P, nbt], f32, name="recip_c")
            lrT_sb = persist.tile([1, b_core], f32, name="lrT_sb")
            rT_sb = persist.tile([1, DIM], f32, name="rT_sb")
            prsT_sb = persist.tile([1, b_core], f32, name="prsT_sb")
            ones8 = persist.tile([P, 2, 16], f8, name="ones8")
            ones_row = persist.tile([1, P], f32, name="ones_row")

            # ---- DRAM: AllReduce bounce (column halves) + transpose scratch ----
            ar_inA = dram.tile([DIM, 512], f8, name="ar_inA")
            ar_inB = dram.tile([DIM, 512], f8, name="ar_inB")
            ar_outA = dram.tile([DIM, 512], f8, name="ar_outA", addr_space="Shared")
            ar_outB = dram.tile([DIM, 512], f8, name="ar_outB", addr_space="Shared")
            sc_lr = dram.tile([nbt, P], f32, name="sc_lr")
            sc_r = dram.tile([NT, P], f32, name="sc_r")
            sc_prs = dram.tile([nbt, P], f32, name="sc_prs")
            warm_in = dram.tile([1, 64], f8, name="warm_in")
            warm_out = dram.tile([1, 64], f8, name="warm_out", addr_space="Shared")

            nc.vector.memset(ones8[:], 1.0)
            nc.vector.memset(ones_row[:], 1.0)

            # warm up the collective stream: the first collective pays a
            # ~50us barrier/setup cost; burn it on a tiny AllReduce that
            # runs concurrently with the startup DMAs.
            wtmp = tmp.tile([1, 64], f8, tag="kv", name="wtmp")
            nc.vector.memset(wtmp[:], 0.0)
            nc.gpsimd.dma_start(out=warm_in[0:1, :], in_=wtmp[:])
            nc.gpsimd.collective_compute(
                "AllReduce",
                mybir.AluOpType.add,
                replica_groups=[list(range(n_cores))],
                ins=[warm_in[:, :]],
                outs=[warm_out[:, :]],
            )

            # ---- small DMAs (gpsimd queue) ----
            nc.gpsimd.dma_start(
                out=bq_c[:],
                in_=bass.AP(tensor=bqs_h, offset=0, ap=[[1, P], [P, NT]]),
            )
            nc.gpsimd.dma_start(
                out=blr_c[:],
                in_=bass.AP(tensor=blr_h, offset=0, ap=[[0, P], [1, 1]]),
            )
            for i in range(NT):
                nc.gpsimd.dma_start(
                    out=wlr16[:, i, :],
                    in_=bass.AP(tensor=wlr16_h, offset=i * P, ap=[[1, P], [P, 1]]),
                )
            # bias broadcasts across partitions via K=1 ones-matmuls
            for bi, (bias_dst, bias_src) in enumerate(
                ((bk_b, bk_h), (wfb_b, wfb_h), (bvcp_b, bvcp_h))
            ):
                for c in range(2):
                    brow = tmp.tile([1, 512], f32, tag="kv", name=f"br{bi}_{c}")
                    nc.gpsimd.dma_start(
                        out=brow[:],
                        in_=bass.AP(tensor=bias_src, offset=c * 512,
                                    ap=[[0, 1], [1, 512]]),
                    )
                    pb = psum.tile([P, 512], f32, tag="ps", name=f"pb{bi}_{c}")
                    nc.tensor.matmul(
                        pb[:], ones_row[:, :], brow[:], start=True, stop=True
                    )
                    nc.vector.tensor_copy(bias_dst[:, c * 512:(c + 1) * 512], pb[:])

            # ---- bulk DMAs: sync queue = q/k weights + xT8; scalar = rest ----
            for i in range(NT):
                nc.sync.dma_start(out=wq8[:, i, :], in_=wq8_h[i * P:(i + 1) * P, :])
            for i in range(NT):
                nc.sync.dma_start(
                    out=xT8a[:, i, :], in_=xT8_h[i * P:(i + 1) * P, 0:b_core // 2]
                )
            for i in range(4):
                nc.sync.dma_start(out=wk8[:, i, :], in_=wk8_h[i * P:(i + 1) * P, :])
            for i in range(4):
                nc.sync.dma_start(out=wk16[:, i, :], in_=wk16_h[i * P:(i + 1) * P, :])
            for i in range(NT):
                nc.sync.dma_start(
                    out=xT8b[:, i, :], in_=xT8_h[i * P:(i + 1) * P, b_core // 2:]
                )
            for t in range(nbt):
                nc.sync.dma_start(out=xn8[:, t, :], in_=xn8_h[t * P:(t + 1) * P, :])
            # scalar queue: xT16 b-chunked (earliest b first), then wv16
            for c in range(nbc):
                for i in range(NT):
                    nc.scalar.dma_start(
                        out=xT16[:, i, c * 512:(c + 1) * 512],
                        in_=xT16_h[i * P:(i + 1) * P, c * 512:(c + 1) * 512],
                    )
            for i in range(NT):
                nc.scalar.dma_start(out=wv16[:, i, :], in_=wv16_h[i * P:(i + 1) * P, :])

            def xt8_lhs(t, j2):
                """fp8 xT lhsT pair slice for global b-tile t, i-pair j2."""
                src = xT8a if t < nbt // 2 else xT8b
                tc_ = t % (nbt // 2)
                return src[:, 2 * j2:2 * j2 + 2, tc_ * P:(tc_ + 1) * P]

            def emit_q(chunks):
                """et8 = exp(qT + bq - SHIFT), transposed layout [h, b]. fp8 DR."""
                for bc in chunks:
                    src = xT8a if bc < nbc // 2 else xT8b
                    lo = (bc % (nbc // 2)) * 512
                    for hb in range(NT):
                        pq = psum.tile([P, 512], f32, tag="ps", name=f"pq{bc}_{hb}")
                        for j in range(NT // 2):
                            nc.tensor.matmul(
                                pq[:],
                                wq8[:, 2 * j:2 * j + 2, hb * P:(hb + 1) * P],
                                src[:, 2 * j:2 * j + 2, lo:lo + 512],
                                start=(j == 0), stop=(j == NT // 2 - 1),
                                perf_mode=DR,
                            )
                        nc.scalar.activation(
                            et8[:, hb, bc * 512:(bc + 1) * 512], pq[:], AF.Exp,
                            bias=bq_c[:, hb:hb + 1], scale=q_act_scale,
                        )

            def emit_lrT():
                """lr_c[p, t] = sigmoid(x @ wlr + blr)[t*128+p] * SG via
                transposed [1, b] matmuls + DRAM bounce."""
                for bc in range(nbc):
                    pl = psmall.tile([1, 512], f32, tag="pl", name=f"plr{bc}")
                    for i in range(NT):
                        nc.tensor.matmul(
                            pl[:],
                            wlr16[:, i, 0:1],
                            xT16[:, i, bc * 512:(bc + 1) * 512],
                            start=(i == 0), stop=(i == NT - 1),
                        )
                    nc.scalar.activation(
                        lrT_sb[0:1, bc * 512:(bc + 1) * 512], pl[:], AF.Sigmoid,
                        bias=blr_c[0:1, 0:1],
                    )
                    nc.gpsimd.dma_start(
                        out=sc_lr[nct * bc:nct * (bc + 1), :],
                        in_=lrT_sb[0:1, bc * 512:(bc + 1) * 512],
                    )
                nc.gpsimd.dma_start(
                    out=lr_c[:, :], in_=sc_lr[:, :].rearrange("a b -> b a")
                )
                nc.vector.tensor_scalar_mul(lr_c[:], lr_c[:], SG)

            def emit_k():
                """g8 = lr * sigmoid(k) * SG, natural layout [b, h].
                Contraction split: i<512 fp8-DR, i>=512 bf16."""
                for t in range(nbt):
                    for c in range(2):
                        pk = psum.tile([P, 512], f32, tag="ps", name=f"pk{t}_{c}")
                        for j2 in range(2):
                            nc.tensor.matmul(
                                pk[:],
                                xt8_lhs(t, j2),
                                wk8[:, 2 * j2:2 * j2 + 2, c * 512:(c + 1) * 512],
                                start=(j2 == 0), stop=False,
                                perf_mode=DR,
                            )
                        for i in range(4):
                            nc.tensor.matmul(
                                pk[:],
                                xT16[:, 4 + i, t * P:(t + 1) * P],
                                wk16[:, i, c * 512:(c + 1) * 512],
                                start=False, stop=(i == 3),
                            )
                        ktmp = tmp.tile([P, 512], f32, tag="kv", name=f"kt{t}_{c}")
                        nc.vector.tensor_add(
                            ktmp[:], pk[:], bk_b[:, c * 512:(c + 1) * 512]
                        )
                        sgk = tmp.tile([P, 512], bf16, tag="sg", name=f"sg{t}_{c}")
                        nc.scalar.activation(sgk[:], ktmp[:], AF.Sigmoid,
                                             scale=k_act_scale)
                        nc.scalar.activation(
                            g8[:, t, c * 512:(c + 1) * 512], sgk[:], AF.Copy,
                            scale=lr_c[:, t:t + 1],
                        )

            def emit_m():
                """mb = x.T @ g (per-core partial), [i, h] layout, fp8 DR;
                then rT = ones.T @ g via [1, 512] matmuls + bounce."""
                for hc in range(2):
                    for ib in range(NT):
                        pm = psum.tile([P, 512], f32, tag="ps", name=f"pm{hc}_{ib}")
                        for bp in range(nbt // 2):
                            nc.tensor.matmul(
                                pm[:],
                                xn8[:, 2 * bp:2 * bp + 2, ib * P:(ib + 1) * P],
                                g8[:, 2 * bp:2 * bp + 2, hc * 512:(hc + 1) * 512],
                                start=(bp == 0), stop=(bp == nbt // 2 - 1),
                                perf_mode=DR,
                            )
                        nc.vector.tensor_copy(
                            mb[:, ib, hc * 512:(hc + 1) * 512], pm[:]
                        )
                for hc in range(2):
                    pr = psmall.tile([1, 512], f32, tag="pl", name=f"pr{hc}")
                    for bt in range(nbt):
                        nc.tensor.matmul(
                            pr[:],
                            ones8[:, 0, 0:1],
                            g8[:, bt, hc * 512:(hc + 1) * 512],
                            start=(bt == 0), stop=(bt == nbt - 1),
                        )
                    nc.vector.tensor_copy(
                        rT_sb[0:1, hc * 512:(hc + 1) * 512], pr[:]
                    )
                    nc.gpsimd.dma_start(
                        out=sc_r[nct * hc:nct * (hc + 1), :],
                        in_=rT_sb[0:1, hc * 512:(hc + 1) * 512],
                    )
                nc.gpsimd.dma_start(
                    out=r_c[:, :], in_=sc_r[:, :].rearrange("a b -> b a")
                )

            def emit_pd(oc):
                """delta.T partial [:, oc half] = mb.T @ wv + r x bvc,
                drained fp8 to one AR column-half."""
                for hb in range(NT):
                    pd = psum.tile([P, 512], f32, tag="ps", name=f"pd{hb}_{oc}")
                    for i in range(NT):
                        nc.tensor.matmul(
                            pd[:],
                            mb[:, i, hb * P:(hb + 1) * P],
                            wv16[:, i, oc * 512:(oc + 1) * 512],
                            start=(i == 0), stop=(i == NT - 1),
                        )
                    pt = tmp.tile([P, 512], f32, tag="kv", name=f"pt{hb}_{oc}")
                    nc.scalar.activation(
                        pt[:], pd[:], AF.Copy, scale=pd_drain_scale
                    )
                    dst = arst.tile([P, 512], f8, tag="ar", name=f"ds{hb}_{oc}")
                    nc.vector.scalar_tensor_tensor(
                        dst[:],
                        bvcp_b[:, oc * 512:(oc + 1) * 512],
                        r_c[:, hb:hb + 1],
                        pt[:],
                        op0=ALU.mult,
                        op1=ALU.add,
                    )
                    ar_dst = ar_inA if oc == 0 else ar_inB
                    eng = nc.sync if oc == 0 else nc.scalar
                    eng.dma_start(
                        out=ar_dst[hb * P:(hb + 1) * P, :], in_=dst[:]
                    )

            def emit_prsT(bc):
                """prsT[b] = sum_h et8[h, b] for one 512-col chunk."""
                pp = psmall.tile([1, 512], f32, tag="pl", name=f"pp{bc}")
                for hb in range(NT):
                    nc.tensor.matmul(
                        pp[:],
                        ones8[:, 0, 0:1],
                        et8[:, hb, bc * 512:(bc + 1) * 512],
                        start=(hb == 0), stop=(hb == NT - 1),
                    )
                nc.vector.tensor_copy(prsT_sb[0:1, bc * 512:(bc + 1) * 512], pp[:])
                nc.gpsimd.dma_start(
                    out=sc_prs[nct * bc:nct * (bc + 1), :],
                    in_=prsT_sb[0:1, bc * 512:(bc + 1) * 512],
                )

            def emit_recip():
                nc.gpsimd.dma_start(
                    out=recip_c[:, :], in_=sc_prs[:, :].rearrange("a b -> b a")
                )
                nc.vector.reciprocal(recip_c[:], recip_c[:])
                nc.vector.tensor_scalar_mul(recip_c[:], recip_c[:], fin_recip_scale)

            def emit_fin(oc, wn):
                """out[:, oc half] = (et8.T @ wn) * recip + wfb, fp8 DR."""
                for t in range(nbt):
                    po = psum.tile([P, 512], f32, tag="ps", name=f"po{t}_{oc}")
                    for j in range(NT // 2):
                        nc.tensor.matmul(
                            po[:],
                            et8[:, 2 * j:2 * j + 2, t * P:(t + 1) * P],
                            wn[:, 2 * j:2 * j + 2, :],
                            start=(j == 0), stop=(j == NT // 2 - 1),
                            perf_mode=DR,
                        )
                    o_st = ost.tile([P, 512], bf16, tag="os", name=f"os{t}_{oc}")
                    nc.vector.scalar_tensor_tensor(
                        o_st[:],
                        po[:],
                        recip_c[:, t:t + 1],
                        wfb_b[:, oc * 512:(oc + 1) * 512],
                        op0=ALU.mult,
                        op1=ALU.add,
                    )
                    eng = nc.sync if oc == 0 else nc.scalar
                    eng.dma_start(
                        out=out_h[t * P:(t + 1) * P, oc * 512:(oc + 1) * 512],
                        in_=o_st[:],
                    )

            # ---- schedule ----
            early_q = list(range(nbc // 2))       # q chunks 0..1 warm up the PE
            late_q = list(range(nbc // 2, nbc))   # the rest hides the AllReduce
            emit_q(early_q)
            emit_lrT()
            emit_k()
            emit_m()
            emit_pd(0)
            nc.gpsimd.collective_compute(
                "AllReduce",
                mybir.AluOpType.add,
                replica_groups=[list(range(n_cores))],
                ins=[ar_inA[:, :]],
                outs=[ar_outA[:, :]],
            )
            emit_pd(1)
            nc.gpsimd.collective_compute(
                "AllReduce",
                mybir.AluOpType.add,
                replica_groups=[list(range(n_cores))],
                ins=[ar_inB[:, :]],
                outs=[ar_outB[:, :]],
            )
            emit_q(late_q)
            for bc in early_q:
                emit_prsT(bc)
            for bc in late_q:
                emit_prsT(bc)
            emit_recip()
            for hb in range(NT):
                nc.sync.dma_start(
                    out=wn8a[:, hb, :], in_=ar_outA[hb * P:(hb + 1) * P, :]
                )
            emit_fin(0, wn8a)
            for hb in range(NT):
                nc.scalar.dma_start(
                    out=wn8b[:, hb, :], in_=ar_outB[hb * P:(hb + 1) * P, :]
                )
            emit_fin(1, wn8b)

    nc.compile()
    return nc


def _host_prep(x, W_slow_w, W_slow_b, W_fast_b, b_core, n_cores):
    """Shard + pre-transpose + cast inputs; returns per-core input maps."""
    Wk = W_slow_w[:DIM]
    Wv = W_slow_w[DIM:2 * DIM]
    Wq = W_slow_w[2 * DIM:3 * DIM]
    wlr = W_slow_w[3 * DIM]

    WkT = np.ascontiguousarray(Wk.T)
    wk8 = np.clip(WkT[:512, :] * SWK, -240.0, 240.0).astype(F8E4)
    wk16 = (WkT[512:, :] * SKP).astype(BF16)
    wv16 = np.ascontiguousarray(Wv.T).astype(BF16)
    wq8 = np.clip(np.ascontiguousarray(Wq.T) * SWQ, -240.0, 240.0).astype(F8E4)
    wlr16 = np.ascontiguousarray(wlr).astype(BF16)

    bk = (W_slow_b[:DIM] * SKP).astype(np.float32)
    b_total = float(b_core * n_cores)
    bvcp = ((W_slow_b[DIM:2 * DIM] - W_fast_b) * (SAR / (b_total * SG))).astype(
        np.float32
    )
    bqs = (W_slow_b[2 * DIM:3 * DIM] - SHIFT).astype(np.float32)
    blr = np.ascontiguousarray(W_slow_b[3 * DIM:3 * DIM + 1]).astype(np.float32)
    wfb = np.ascontiguousarray(W_fast_b).astype(np.float32)

    in_maps = []
    for c in range(n_cores):
        xs = x[c * b_core:(c + 1) * b_core, :]
        xT = np.ascontiguousarray(xs.T)
        xT16 = xT.astype(BF16)
        xT8 = np.clip(xT * SX, -240.0, 240.0).astype(F8E4)
        xn8 = np.clip(xs * SX, -240.0, 240.0).astype(F8E4)
        in_maps.append({
            "xT16": xT16, "xT8": xT8, "xn8": np.ascontiguousarray(xn8),
            "wk8": wk8, "wk16": wk16, "wq8": wq8, "wv16": wv16, "wlr16": wlr16,
            "bk": bk, "bqs": bqs, "blr": blr, "bvcp": bvcp, "wfb": wfb,
        })
    return in_maps


_PROGRAM_CACHE = {}


def _get_program(b_core, n_cores=N_CORES):
    key = (b_core, n_cores)
    if key not in _PROGRAM_CACHE:
        _PROGRAM_CACHE[key] = _build_program(b_core, n_cores)
    return _PROGRAM_CACHE[key]


def _run_device(x, W_slow_w, W_slow_b, W_fast_b, trace=False):
    from concourse.bass_utils import run_bass_kernel_spmd

    b_core = x.shape[0] // N_CORES
    nc = _get_program(b_core)
    in_maps = _host_prep(x, W_slow_w, W_slow_b, W_fast_b, b_core, N_CORES)
    res = run_bass_kernel_spmd(nc, in_maps, list(range(N_CORES)), trace=trace)
    out = np.concatenate([res.results[c]["out"] for c in range(N_CORES)], axis=0)
    return out.astype(np.float32), res


def _reference_numpy(x, W_slow_w, W_slow_b, W_fast_w, W_fast_b):
    """Exact fallback (only used if W_fast_w != 0, which the spec never produces)."""
    x = x.astype(np.float64)
    s = x @ W_slow_w.astype(np.float64).T + W_slow_b.astype(np.float64)
    k = s[:, :DIM]
    v = s[:, DIM:2 * DIM]
    q = s[:, 2 * DIM:3 * DIM]
    lr = 1.0 / (1.0 + np.exp(-s[:, -1:]))
    ek = np.exp(k - k.max(axis=1, keepdims=True))
    ak = ek / ek.sum(axis=1, keepdims=True)
    v_bar = ak @ W_fast_w.astype(np.float64).T + W_fast_b.astype(np.float64)
    sigk = 1.0 / (1.0 + np.exp(-k))
    delta = (lr * (v - v_bar)).T @ sigk / x.shape[0]
    w_new = W_fast_w.astype(np.float64) + delta
    eq = np.exp(q - q.max(axis=1, keepdims=True))
    aq = eq / eq.sum(axis=1, keepdims=True)
    return (aq @ w_new.T + W_fast_b.astype(np.float64)).astype(np.float32)


def kernel(x, W_slow_w, W_slow_b, W_fast_w, W_fast_b):
    x = np.asarray(x)
    W_slow_w = np.asarray(W_slow_w)
    W_slow_b = np.asarray(W_slow_b)
    W_fast_w = np.asarray(W_fast_w)
    W_fast_b = np.asarray(W_fast_b)
    if np.any(W_fast_w):
        # Spec guarantees W_fast_w == 0; exact fallback for generality.
        return _reference_numpy(x, W_slow_w, W_slow_b, W_fast_w, W_fast_b)
    out, _ = _run_device(x, W_slow_w, W_slow_b, W_fast_b, trace=False)
    return out


# revision 19
# speedup vs baseline: 1.3150x; 1.0481x over previous
"""DeltaNet fused kernel for 8 TRN2 NeuronCores (Bass/Tile), fp8-hybrid v2.

Math (reference, with W_fast_w == 0 so v_bar == W_fast_b):
    s  = x @ W_slow_w.T + W_slow_b            [B, 3073]
    k  = s[:, :1024]; v = s[:, 1024:2048]; q = s[:, 2048:3072]
    lr = sigmoid(s[:, 3072])
    delta[o,h] = sum_b (lr*(v - wfb))[b,o] * sigmoid(k)[b,h] / B
    out = softmax(q) @ delta.T + wfb

Restructured to eliminate the v projection (v = x @ Wv.T + bv):
    g  = lr * sigmoid(k)                      [B, H]
    M  = x.T @ g                              [I, H]   (per-core partial)
    r  = sum_b g[b, :]                        [H]
    delta.T = (M.T @ Wv.T + r x (bv - wfb)) / B        [H, O]  (AllReduced)
    out = softmax(q) @ delta.T + wfb

Precision (validated in numpy simulation; end-to-end rel err 6.6e-3 vs 2e-2
tolerance): fp8-e4m3 DoubleRow for q / M / final matmuls and the first half
of the k contraction; bf16 for the rest of k, lr, and the delta.T matmul.
The AllReduce runs in fp8 (delta scaled by 4096), split into two [H, 512]
column-halves so the final matmul's first half can start after the first AR.

Schedule: q chunk 0 warms up the PE while the k operands stream in; the
whole rest of the q-phase plus the softmax row-sums run after the AR
trigger to hide the collective; the final matmul is split into per-AR-half
passes. lr / r / rowsum are computed as transposed [1, N] matmuls (cheap
N=512 streams instead of 384 N=1 matmuls) and moved cross-partition via
tiny DRAM round-trips.
"""

import os
import sys

for _p in ("/opt/trn_rl_repo", "/root/.axon_site/_ro/trn_rl_repo"):
    if os.path.isdir(_p) and _p not in sys.path:
        sys.path.append(_p)

import numpy as np
import ml_dtypes

BF16 = ml_dtypes.bfloat16
F8E4 = ml_dtypes.float8_e4m3     # TRN fp8e4: max normal +-240

N_CORES = 8
B_FULL = 16384
DIM = 1024          # dim_in == dim_out == dim_hidden
P = 128
NT = DIM // P       # 8 tiles along any 1024 dim

SX = 16.0           # x fp8 scale             (|x|max 5.4  -> 87)
SWQ = 512.0         # Wq fp8 scale            (|w|max .16  -> 80)
SWK = 512.0         # Wk fp8 scale (i < 512 half)
SKP = SX * SWK      # k psum scale (bf16 half pre-scaled to match)
SWL = 512.0         # wlr fp8 scale (i < 512 half)
SG = 128.0          # g fp8 scale             (g in (0,1)  -> <128)
SAR = 4096.0        # delta fp8 scale         (|delta|max .043 -> 176)
SHIFT = 3.0         # exp shift               (max q+bq 7.63 -> et < 103)


def _build_program(b_core: int, n_cores: int = N_CORES):
    """Build the SPMD Bass program (same program on every core)."""
    import concourse.bass as bass
    import concourse.mybir as mybir
    import concourse.tile as tile
    from concourse import bacc

    f32 = mybir.dt.float32
    bf16 = mybir.dt.bfloat16
    f8 = mybir.dt.float8e4
    AF = mybir.ActivationFunctionType
    ALU = mybir.AluOpType
    DR = mybir.MatmulPerfMode.DoubleRow

    nbt = b_core // P               # b-tiles per core (16)
    nbc = b_core // 512             # 512-wide b-chunks (4)
    nct = 512 // P                  # b-tiles per chunk (4)
    assert b_core % 1024 == 0

    nc = bacc.Bacc(
        "TRN2",
        target_bir_lowering=False,
        debug=False,
        num_devices=n_cores,
    )

    # ---- kernel I/O ----
    xT16_h = nc.dram_tensor("xT16", [512, b_core], bf16, kind="ExternalInput")
    xT8_h = nc.dram_tensor("xT8", [DIM, b_core], f8, kind="ExternalInput")
    xn8_h = nc.dram_tensor("xn8", [b_core, DIM], f8, kind="ExternalInput")
    wk8_h = nc.dram_tensor("wk8", [512, DIM], f8, kind="ExternalInput")
    wk16_h = nc.dram_tensor("wk16", [512, DIM], bf16, kind="ExternalInput")
    wq8_h = nc.dram_tensor("wq8", [DIM, DIM], f8, kind="ExternalInput")
    wv16_h = nc.dram_tensor("wv16", [DIM, DIM], bf16, kind="ExternalInput")
    wlr8_h = nc.dram_tensor("wlr8", [512], f8, kind="ExternalInput")
    wlr16_h = nc.dram_tensor("wlr16", [512], bf16, kind="ExternalInput")
    bk_h = nc.dram_tensor("bk", [DIM], f32, kind="ExternalInput")     # bk * SKP
    bqs_h = nc.dram_tensor("bqs", [DIM], f32, kind="ExternalInput")   # bq - SHIFT
    blr_h = nc.dram_tensor("blr", [1], f32, kind="ExternalInput")
    # (bv - wfb) * SAR / (b_total * SG): outer-product operand for the drain
    bvcp_h = nc.dram_tensor("bvcp", [DIM], f32, kind="ExternalInput")
    wfb_h = nc.dram_tensor("wfb", [DIM], f32, kind="ExternalInput")
    out_h = nc.dram_tensor("out", [b_core, DIM], bf16, kind="ExternalOutput")

    pd_drain_scale = SAR / (float(b_core * n_cores) * SX * SG)
    fin_recip_scale = 1.0 / SAR
    q_act_scale = 1.0 / (SX * SWQ)
    k_act_scale = 1.0 / SKP
    lr_act_scale = 1.0 / (SX * SWL)

    with tile.TileContext(nc) as tc:
        with (
            tc.tile_pool(name="persist", bufs=1) as persist,
            tc.tile_pool(name="psum", bufs=6, space="PSUM") as psum,
            tc.tile_pool(name="psmall", bufs=2, space="PSUM") as psmall,
            tc.tile_pool(name="tmp", bufs=4) as tmp,
            tc.tile_pool(name="ost", bufs=6) as ost,
            tc.tile_pool(name="arst", bufs=4) as arst,
            tc.tile_pool(name="dram", bufs=1, space="DRAM") as dram,
        ):
            # ---- persistent SBUF tensors ----
            wq8 = persist.tile([P, NT, DIM], f8, name="wq8")
            xT8a = persist.tile([P, NT, b_core // 2], f8, name="xT8a")
            xT8b = persist.tile([P, NT, b_core // 2], f8, name="xT8b")
            wk8 = persist.tile([P, 4, DIM], f8, name="wk8")
            wk16 = persist.tile([P, 4, DIM], bf16, name="wk16")
            wlr8 = persist.tile([P, 4, 1], f8, name="wlr8")
            wlr16 = persist.tile([P, 4, 1], bf16, name="wlr16")
            xT16 = persist.tile([P, 4, b_core], bf16, name="xT16")
            xn8 = persist.tile([P, nbt, DIM], f8, name="xn8")
            wv16 = persist.tile([P, NT, DIM], bf16, name="wv16")
            g8 = persist.tile([P, nbt, DIM], f8, name="g8")
            et8 = persist.tile([P, NT, b_core], f8, name="et8")
            mb = persist.tile([P, NT, DIM], bf16, name="mb")
            wn8a = persist.tile([P, NT, 512], f8, name="wn8a")
            wn8b = persist.tile([P, NT, 512], f8, name="wn8b")
            bk_b = persist.tile([P, DIM], f32, name="bk_b")
            wfb_b = persist.tile([P, DIM], f32, name="wfb_b")
            bvcp_b = persist.tile([P, DIM], f32, name="bvcp_b")
            bq_c = persist.tile([P, NT], f32, name="bq_c")
            blr_c = persist.tile([P, 1], f32, name="blr_c")
            lr_c = persist.tile([P, nbt], f32, name="lr_c")
            r_c = persist.tile([P, NT], f32, name="r_c")
            recip_c = persist.tile([P, nbt], f32, name="recip_c")
            lrT_sb = persist.tile([1, b_core], f32, name="lrT_sb")
            rT_sb = persist.tile([1, DIM], f32, name="rT_sb")
            prsT_sb = persist.tile([1, b_core], f32, name="prsT_sb")
            ones8 = persist.tile([P, 2, 16], f8, name="ones8")
            ones_row = persist.tile([1, P], f32, name="ones_row")

            # ---- DRAM: AllReduce bounce (column halves) + transpose scratch ----
            ar_inA = dram.tile([DIM, 512], f8, name="ar_inA")
            ar_inB = dram.tile([DIM, 512], f8, name="ar_inB")
            ar_outA = dram.tile([DIM, 512], f8, name="ar_outA", addr_space="Shared")
            ar_outB = dram.tile([DIM, 512], f8, name="ar_outB", addr_space="Shared")
            sc_lr = dram.tile([nbt, P], f32, name="sc_lr")
            sc_r = dram.tile([NT, P], f32, name="sc_r")
            sc_prs = dram.tile([nbt, P], f32, name="sc_prs")
            warm_in = dram.tile([1, 64], f8, name="warm_in")
            warm_out = dram.tile([1, 64], f8, name="warm_out", addr_space="Shared")

            nc.vector.memset(ones8[:], 1.0)
            nc.vector.memset(ones_row[:], 1.0)

            # warm up the collective stream: the first collective pays a
            # ~50us barrier/setup cost; burn it on a tiny AllReduce that
            # runs concurrently with the startup DMAs.
            wtmp = tmp.tile([1, 64], f8, tag="kv", name="wtmp")
            nc.vector.memset(wtmp[:], 0.0)
            nc.gpsimd.dma_start(out=warm_in[0:1, :], in_=wtmp[:])
            nc.gpsimd.collective_compute(
                "AllReduce",
                mybir.AluOpType.add,
                replica_groups=[list(range(n_cores))],
                ins=[warm_in[:, :]],
                outs=[warm_out[:, :]],
            )

            # ---- small DMAs (gpsimd queue) ----
            nc.gpsimd.dma_start(
                out=bq_c[:],
                in_=bass.AP(tensor=bqs_h, offset=0, ap=[[1, P], [P, NT]]),
            )
            nc.gpsimd.dma_start(
                out=blr_c[:],
                in_=bass.AP(tensor=blr_h, offset=0, ap=[[0, P], [1, 1]]),
            )
            for i in range(4):
                nc.gpsimd.dma_start(
                    out=wlr8[:, i, :],
                    in_=bass.AP(tensor=wlr8_h, offset=i * P, ap=[[1, P], [P, 1]]),
                )
                nc.gpsimd.dma_start(
                    out=wlr16[:, i, :],
                    in_=bass.AP(tensor=wlr16_h, offset=i * P, ap=[[1, P], [P, 1]]),
                )
            # bias broadcasts across partitions via K=1 ones-matmuls
            for bi, (bias_dst, bias_src) in enumerate(
                ((bk_b, bk_h), (wfb_b, wfb_h), (bvcp_b, bvcp_h))
            ):
                for c in range(2):
                    brow = tmp.tile([1, 512], f32, tag="kv", name=f"br{bi}_{c}")
                    nc.gpsimd.dma_start(
                        out=brow[:],
                        in_=bass.AP(tensor=bias_src, offset=c * 512,
                                    ap=[[0, 1], [1, 512]]),
                    )
                    pb = psum.tile([P, 512], f32, tag="ps", name=f"pb{bi}_{c}")
                    nc.tensor.matmul(
                        pb[:], ones_row[:, :], brow[:], start=True, stop=True
                    )
                    nc.vector.tensor_copy(bias_dst[:, c * 512:(c + 1) * 512], pb[:])

            # ---- bulk DMAs: sync queue = q/k weights + xT8; scalar = rest ----
            for i in range(NT):
                nc.sync.dma_start(out=wq8[:, i, :], in_=wq8_h[i * P:(i + 1) * P, :])
            for i in range(NT):
                nc.sync.dma_start(
                    out=xT8a[:, i, :], in_=xT8_h[i * P:(i + 1) * P, 0:b_core // 2]
                )
            for i in range(4):
                nc.sync.dma_start(out=wk8[:, i, :], in_=wk8_h[i * P:(i + 1) * P, :])
            for i in range(4):
                nc.sync.dma_start(out=wk16[:, i, :], in_=wk16_h[i * P:(i + 1) * P, :])
            for i in range(NT):
                nc.sync.dma_start(
                    out=xT8b[:, i, :], in_=xT8_h[i * P:(i + 1) * P, b_core // 2:]
                )
            # scalar queue: xT16 b-chunked (earliest b first), then xn8, wv16
            for c in range(nbc):
                for i in range(4):
                    nc.scalar.dma_start(
                        out=xT16[:, i, c * 512:(c + 1) * 512],
                        in_=xT16_h[i * P:(i + 1) * P, c * 512:(c + 1) * 512],
                    )
            for t in range(nbt):
                nc.scalar.dma_start(out=xn8[:, t, :], in_=xn8_h[t * P:(t + 1) * P, :])
            for i in range(NT):
                nc.scalar.dma_start(out=wv16[:, i, :], in_=wv16_h[i * P:(i + 1) * P, :])

            def xt8_lhs(t, j2):
                """fp8 xT lhsT pair slice for global b-tile t, i-pair j2."""
                src = xT8a if t < nbt // 2 else xT8b
                tc_ = t % (nbt // 2)
                return src[:, 2 * j2:2 * j2 + 2, tc_ * P:(tc_ + 1) * P]

            def emit_q(chunks):
                """et8 = exp(qT + bq - SHIFT), transposed layout [h, b]. fp8 DR."""
                for bc in chunks:
                    src = xT8a if bc < nbc // 2 else xT8b
                    lo = (bc % (nbc // 2)) * 512
                    for hb in range(NT):
                        pq = psum.tile([P, 512], f32, tag="ps", name=f"pq{bc}_{hb}")
                        for j in range(NT // 2):
                            nc.tensor.matmul(
                                pq[:],
                                wq8[:, 2 * j:2 * j + 2, hb * P:(hb + 1) * P],
                                src[:, 2 * j:2 * j + 2, lo:lo + 512],
                                start=(j == 0), stop=(j == NT // 2 - 1),
                                perf_mode=DR,
                            )
                        nc.scalar.activation(
                            et8[:, hb, bc * 512:(bc + 1) * 512], pq[:], AF.Exp,
                            bias=bq_c[:, hb:hb + 1], scale=q_act_scale,
                        )

            def emit_lrT():
                """lr_c[p, t] = sigmoid(x @ wlr + blr)[t*128+p] * SG via
                transposed [1, b] matmuls + DRAM bounce."""
                for bc in range(nbc):
                    src8 = xT8a if bc < nbc // 2 else xT8b
                    lo = (bc % (nbc // 2)) * 512
                    pl = psmall.tile([1, 512], f32, tag="pl", name=f"plr{bc}")
                    for i in range(4):
                        nc.tensor.matmul(
                            pl[:],
                            wlr8[:, i, 0:1],
                            src8[:, i, lo:lo + 512],
                            start=(i == 0), stop=False,
                        )
                    for i in range(4):
                        nc.tensor.matmul(
                            pl[:],
                            wlr16[:, i, 0:1],
                            xT16[:, i, bc * 512:(bc + 1) * 512],
                            start=False, stop=(i == 3),
                        )
                    nc.scalar.activation(
                        lrT_sb[0:1, bc * 512:(bc + 1) * 512], pl[:], AF.Sigmoid,
                        bias=blr_c[0:1, 0:1], scale=lr_act_scale,
                    )
                    nc.gpsimd.dma_start(
                        out=sc_lr[nct * bc:nct * (bc + 1), :],
                        in_=lrT_sb[0:1, bc * 512:(bc + 1) * 512],
                    )
                nc.gpsimd.dma_start(
                    out=lr_c[:, :], in_=sc_lr[:, :].rearrange("a b -> b a")
                )
                nc.vector.tensor_scalar_mul(lr_c[:], lr_c[:], SG)

            def emit_k():
                """g8 = lr * sigmoid(k) * SG, natural layout [b, h].
                Contraction split: i<512 fp8-DR, i>=512 bf16."""
                for t in range(nbt):
                    for c in range(2):
                        pk = psum.tile([P, 512], f32, tag="ps", name=f"pk{t}_{c}")
                        for j2 in range(2):
                            nc.tensor.matmul(
                                pk[:],
                                xt8_lhs(t, j2),
                                wk8[:, 2 * j2:2 * j2 + 2, c * 512:(c + 1) * 512],
                                start=(j2 == 0), stop=False,
                                perf_mode=DR,
                            )
                        for i in range(4):
                            nc.tensor.matmul(
                                pk[:],
                                xT16[:, i, t * P:(t + 1) * P],
                                wk16[:, i, c * 512:(c + 1) * 512],
                                start=False, stop=(i == 3),
                            )
                        ktmp = tmp.tile([P, 512], f32, tag="kv", name=f"kt{t}_{c}")
                        nc.vector.tensor_add(
                            ktmp[:], pk[:], bk_b[:, c * 512:(c + 1) * 512]
                        )
                        sgk = tmp.tile([P, 512], bf16, tag="sg", name=f"sg{t}_{c}")
                        nc.scalar.activation(sgk[:], ktmp[:], AF.Sigmoid,
                                             scale=k_act_scale)
                        nc.scalar.activation(
                            g8[:, t, c * 512:(c + 1) * 512], sgk[:], AF.Copy,
                            scale=lr_c[:, t:t + 1],
                        )

            def emit_m():
                """mb = x.T @ g (per-core partial), [i, h] layout, fp8 DR;
                then rT = ones.T @ g via [1, 512] matmuls + bounce."""
                for hc in range(2):
                    for ib in range(NT):
                        pm = psum.tile([P, 512], f32, tag="ps", name=f"pm{hc}_{ib}")
                        for bp in range(nbt // 2):
                            nc.tensor.matmul(
                                pm[:],
                                xn8[:, 2 * bp:2 * bp + 2, ib * P:(ib + 1) * P],
                                g8[:, 2 * bp:2 * bp + 2, hc * 512:(hc + 1) * 512],
                                start=(bp == 0), stop=(bp == nbt // 2 - 1),
                                perf_mode=DR,
                            )
                        nc.vector.tensor_copy(
                            mb[:, ib, hc * 512:(hc + 1) * 512], pm[:]
                        )
                for hc in range(2):
                    pr = psmall.tile([1, 512], f32, tag="pl", name=f"pr{hc}")
                    for bt in range(nbt):
                        nc.tensor.matmul(
                            pr[:],
                            ones8[:, 0, 0:1],
                            g8[:, bt, hc * 512:(hc + 1) * 512],
                            start=(bt == 0), stop=(bt == nbt - 1),
                        )
                    nc.vector.tensor_copy(
                        rT_sb[0:1, hc * 512:(hc + 1) * 512], pr[:]
                    )
                    nc.gpsimd.dma_start(
                        out=sc_r[nct * hc:nct * (hc + 1), :],
                        in_=rT_sb[0:1, hc * 512:(hc + 1) * 512],
                    )
                nc.gpsimd.dma_start(
                    out=r_c[:, :], in_=sc_r[:, :].rearrange("a b -> b a")
                )

            def emit_pd(oc):
                """delta.T partial [:, oc half] = mb.T @ wv + r x bvc,
                drained fp8 to one AR column-half."""
                for hb in range(NT):
                    pd = psum.tile([P, 512], f32, tag="ps", name=f"pd{hb}_{oc}")
                    for i in range(NT):
                        nc.tensor.matmul(
                            pd[:],
                            mb[:, i, hb * P:(hb + 1) * P],
                            wv16[:, i, oc * 512:(oc + 1) * 512],
                            start=(i == 0), stop=(i == NT - 1),
                        )
                    pt = tmp.tile([P, 512], f32, tag="kv", name=f"pt{hb}_{oc}")
                    nc.scalar.activation(
                        pt[:], pd[:], AF.Copy, scale=pd_drain_scale
                    )
                    dst = arst.tile([P, 512], f8, tag="ar", name=f"ds{hb}_{oc}")
                    nc.vector.scalar_tensor_tensor(
                        dst[:],
                        bvcp_b[:, oc * 512:(oc + 1) * 512],
                        r_c[:, hb:hb + 1],
                        pt[:],
                        op0=ALU.mult,
                        op1=ALU.add,
                    )
                    ar_dst = ar_inA if oc == 0 else ar_inB
                    eng = nc.sync if oc == 0 else nc.scalar
                    eng.dma_start(
                        out=ar_dst[hb * P:(hb + 1) * P, :], in_=dst[:]
                    )

            def emit_prsT(bc):
                """prsT[b] = sum_h et8[h, b] for one 512-col chunk."""
                pp = psmall.tile([1, 512], f32, tag="pl", name=f"pp{bc}")
                for hb in range(NT):
                    nc.tensor.matmul(
                        pp[:],
                        ones8[:, 0, 0:1],
                        et8[:, hb, bc * 512:(bc + 1) * 512],
                        start=(hb == 0), stop=(hb == NT - 1),
                    )
                nc.vector.tensor_copy(prsT_sb[0:1, bc * 512:(bc + 1) * 512], pp[:])
                nc.sync.dma_start(
                    out=sc_prs[nct * bc:nct * (bc + 1), :],
                    in_=prsT_sb[0:1, bc * 512:(bc + 1) * 512],
                )

            def emit_recip():
                nc.sync.dma_start(
                    out=recip_c[:, :], in_=sc_prs[:, :].rearrange("a b -> b a")
                )
                nc.vector.reciprocal(recip_c[:], recip_c[:])
                nc.vector.tensor_scalar_mul(recip_c[:], recip_c[:], fin_recip_scale)

            def emit_fin(oc, wn):
                """out[:, oc half] = (et8.T @ wn) * recip + wfb, fp8 DR."""
                for t in range(nbt):
                    po = psum.tile([P, 512], f32, tag="ps", name=f"po{t}_{oc}")
                    for j in range(NT // 2):
                        nc.tensor.matmul(
                            po[:],
                            et8[:, 2 * j:2 * j + 2, t * P:(t + 1) * P],
                            wn[:, 2 * j:2 * j + 2, :],
                            start=(j == 0), stop=(j == NT // 2 - 1),
                            perf_mode=DR,
                        )
                    o_st = ost.tile([P, 512], bf16, tag="os", name=f"os{t}_{oc}")
                    nc.vector.scalar_tensor_tensor(
                        o_st[:],
                        po[:],
                        recip_c[:, t:t + 1],
                        wfb_b[:, oc * 512:(oc + 1) * 512],
                        op0=ALU.mult,
                        op1=ALU.add,
                    )
                    eng = nc.sync if oc == 0 else nc.scalar
                    eng.dma_start(
                        out=out_h[t * P:(t + 1) * P, oc * 512:(oc + 1) * 512],
                        in_=o_st[:],
                    )

            # ---- schedule ----
            early_q = list(range(nbc // 2))       # q chunks 0..1 warm up the PE
            late_q = list(range(nbc // 2, nbc))   # the rest hides the AllReduce
            emit_q(early_q)
            emit_lrT()
            emit_k()
            emit_m()
            emit_pd(0)
            nc.gpsimd.collective_compute(
                "AllReduce",
                mybir.AluOpType.add,
                replica_groups=[list(range(n_cores))],
                ins=[ar_inA[:, :]],
                outs=[ar_outA[:, :]],
            )
            emit_pd(1)
            nc.gpsimd.collective_compute(
                "AllReduce",
                mybir.AluOpType.add,
                replica_groups=[list(range(n_cores))],
                ins=[ar_inB[:, :]],
                outs=[ar_outB[:, :]],
            )
            emit_q(late_q)
            for bc in early_q:
                emit_prsT(bc)
            for bc in late_q:
                emit_prsT(bc)
            emit_recip()
            for hb in range(NT):
                nc.sync.dma_start(
                    out=wn8a[:, hb, :], in_=ar_outA[hb * P:(hb + 1) * P, :]
                )
            emit_fin(0, wn8a)
            for hb in range(NT):
                nc.scalar.dma_start(
                    out=wn8b[:, hb, :], in_=ar_outB[hb * P:(hb + 1) * P, :]
                )
            emit_fin(1, wn8b)

    nc.compile()
    return nc


def _host_prep(x, W_slow_w, W_slow_b, W_fast_b, b_core, n_cores):
    """Shard + pre-transpose + cast inputs; returns per-core input maps."""
    Wk = W_slow_w[:DIM]
    Wv = W_slow_w[DIM:2 * DIM]
    Wq = W_slow_w[2 * DIM:3 * DIM]
    wlr = W_slow_w[3 * DIM]

    WkT = np.ascontiguousarray(Wk.T)
    wk8 = np.clip(WkT[:512, :] * SWK, -240.0, 240.0).astype(F8E4)
    wk16 = (WkT[512:, :] * SKP).astype(BF16)
    wv16 = np.ascontiguousarray(Wv.T).astype(BF16)
    wq8 = np.clip(np.ascontiguousarray(Wq.T) * SWQ, -240.0, 240.0).astype(F8E4)
    wlr8 = np.clip(wlr[:512] * SWL, -240.0, 240.0).astype(F8E4)
    wlr16 = (wlr[512:] * (SX * SWL)).astype(BF16)

    bk = (W_slow_b[:DIM] * SKP).astype(np.float32)
    b_total = float(b_core * n_cores)
    bvcp = ((W_slow_b[DIM:2 * DIM] - W_fast_b) * (SAR / (b_total * SG))).astype(
        np.float32
    )
    bqs = (W_slow_b[2 * DIM:3 * DIM] - SHIFT).astype(np.float32)
    blr = np.ascontiguousarray(W_slow_b[3 * DIM:3 * DIM + 1]).astype(np.float32)
    wfb = np.ascontiguousarray(W_fast_b).astype(np.float32)

    in_maps = []
    for c in range(n_cores):
        xs = x[c * b_core:(c + 1) * b_core, :]
        xT = np.ascontiguousarray(xs.T)
        xT16 = np.ascontiguousarray(xT[512:]).astype(BF16)
        xT8 = np.clip(xT * SX, -240.0, 240.0).astype(F8E4)
        xn8 = np.clip(xs * SX, -240.0, 240.0).astype(F8E4)
        in_maps.append({
            "xT16": xT16, "xT8": xT8, "xn8": np.ascontiguousarray(xn8),
            "wk8": wk8, "wk16": wk16, "wq8": wq8, "wv16": wv16,
            "wlr8": wlr8, "wlr16": wlr16,
            "bk": bk, "bqs": bqs, "blr": blr, "bvcp": bvcp, "wfb": wfb,
        })
    return in_maps


_PROGRAM_CACHE = {}


def _get_program(b_core, n_cores=N_CORES):
    key = (b_core, n_cores)
    if key not in _PROGRAM_CACHE:
        _PROGRAM_CACHE[key] = _build_program(b_core, n_cores)
    return _PROGRAM_CACHE[key]


def _run_device(x, W_slow_w, W_slow_b, W_fast_b, trace=False):
    from concourse.bass_utils import run_bass_kernel_spmd

    b_core = x.shape[0] // N_CORES
    nc = _get_program(b_core)
    in_maps = _host_prep(x, W_slow_w, W_slow_b, W_fast_b, b_core, N_CORES)
    res = run_bass_kernel_spmd(nc, in_maps, list(range(N_CORES)), trace=trace)
    out = np.concatenate([res.results[c]["out"] for c in range(N_CORES)], axis=0)
    return out.astype(np.float32), res


def _reference_numpy(x, W_slow_w, W_slow_b, W_fast_w, W_fast_b):
    """Exact fallback (only used if W_fast_w != 0, which the spec never produces)."""
    x = x.astype(np.float64)
    s = x @ W_slow_w.astype(np.float64).T + W_slow_b.astype(np.float64)
    k = s[:, :DIM]
    v = s[:, DIM:2 * DIM]
    q = s[:, 2 * DIM:3 * DIM]
    lr = 1.0 / (1.0 + np.exp(-s[:, -1:]))
    ek = np.exp(k - k.max(axis=1, keepdims=True))
    ak = ek / ek.sum(axis=1, keepdims=True)
    v_bar = ak @ W_fast_w.astype(np.float64).T + W_fast_b.astype(np.float64)
    sigk = 1.0 / (1.0 + np.exp(-k))
    delta = (lr * (v - v_bar)).T @ sigk / x.shape[0]
    w_new = W_fast_w.astype(np.float64) + delta
    eq = np.exp(q - q.max(axis=1, keepdims=True))
    aq = eq / eq.sum(axis=1, keepdims=True)
    return (aq @ w_new.T + W_fast_b.astype(np.float64)).astype(np.float32)


def kernel(x, W_slow_w, W_slow_b, W_fast_w, W_fast_b):
    x = np.asarray(x)
    W_slow_w = np.asarray(W_slow_w)
    W_slow_b = np.asarray(W_slow_b)
    W_fast_w = np.asarray(W_fast_w)
    W_fast_b = np.asarray(W_fast_b)
    if np.any(W_fast_w):
        # Spec guarantees W_fast_w == 0; exact fallback for generality.
        return _reference_numpy(x, W_slow_w, W_slow_b, W_fast_w, W_fast_b)
    out, _ = _run_device(x, W_slow_w, W_slow_b, W_fast_b, trace=False)
    return out


# revision 21
# speedup vs baseline: 1.3513x; 1.0276x over previous
"""DeltaNet fused kernel for 8 TRN2 NeuronCores (Bass/Tile), fp8-hybrid v2.

Math (reference, with W_fast_w == 0 so v_bar == W_fast_b):
    s  = x @ W_slow_w.T + W_slow_b            [B, 3073]
    k  = s[:, :1024]; v = s[:, 1024:2048]; q = s[:, 2048:3072]
    lr = sigmoid(s[:, 3072])
    delta[o,h] = sum_b (lr*(v - wfb))[b,o] * sigmoid(k)[b,h] / B
    out = softmax(q) @ delta.T + wfb

Restructured to eliminate the v projection (v = x @ Wv.T + bv):
    g  = lr * sigmoid(k)                      [B, H]
    M  = x.T @ g                              [I, H]   (per-core partial)
    r  = sum_b g[b, :]                        [H]
    delta.T = (M.T @ Wv.T + r x (bv - wfb)) / B        [H, O]  (AllReduced)
    out = softmax(q) @ delta.T + wfb

Precision (validated in numpy simulation; end-to-end rel err 6.6e-3 vs 2e-2
tolerance): fp8-e4m3 DoubleRow for q / M / final matmuls and the first half
of the k contraction; bf16 for the rest of k, lr, and the delta.T matmul.
The AllReduce runs in fp8 (delta scaled by 4096), split into two [H, 512]
column-halves so the final matmul's first half can start after the first AR.

Schedule: q chunk 0 warms up the PE while the k operands stream in; the
whole rest of the q-phase plus the softmax row-sums run after the AR
trigger to hide the collective; the final matmul is split into per-AR-half
passes. lr / r / rowsum are computed as transposed [1, N] matmuls (cheap
N=512 streams instead of 384 N=1 matmuls) and moved cross-partition via
tiny DRAM round-trips.
"""

import os
import sys

for _p in ("/opt/trn_rl_repo", "/root/.axon_site/_ro/trn_rl_repo"):
    if os.path.isdir(_p) and _p not in sys.path:
        sys.path.append(_p)

import numpy as np
import ml_dtypes

BF16 = ml_dtypes.bfloat16
F8E4 = ml_dtypes.float8_e4m3     # TRN fp8e4: max normal +-240

N_CORES = 8
B_FULL = 16384
DIM = 1024          # dim_in == dim_out == dim_hidden
P = 128
NT = DIM // P       # 8 tiles along any 1024 dim

SX = 16.0           # x fp8 scale             (|x|max 5.4  -> 87)
SWQ = 512.0         # Wq fp8 scale            (|w|max .16  -> 80)
SWK = 512.0         # Wk fp8 scale (i < 512 half)
SKP = SX * SWK      # k psum scale (bf16 half pre-scaled to match)
SWL = 512.0         # wlr fp8 scale (i < 512 half)
SG = 128.0          # g fp8 scale             (g in (0,1)  -> <128)
SAR = 4096.0        # delta fp8 scale         (|delta|max .043 -> 176)
SHIFT = 3.0         # exp shift               (max q+bq 7.63 -> et < 103)


def _build_program(b_core: int, n_cores: int = N_CORES):
    """Build the SPMD Bass program (same program on every core)."""
    import concourse.bass as bass
    import concourse.mybir as mybir
    import concourse.tile as tile
    from concourse import bacc

    f32 = mybir.dt.float32
    bf16 = mybir.dt.bfloat16
    f8 = mybir.dt.float8e4
    AF = mybir.ActivationFunctionType
    ALU = mybir.AluOpType
    DR = mybir.MatmulPerfMode.DoubleRow

    nbt = b_core // P               # b-tiles per core (16)
    nbc = b_core // 512             # 512-wide b-chunks (4)
    nct = 512 // P                  # b-tiles per chunk (4)
    assert b_core % 1024 == 0

    nc = bacc.Bacc(
        "TRN2",
        target_bir_lowering=False,
        debug=False,
        num_devices=n_cores,
    )

    # ---- kernel I/O ----
    xT16_h = nc.dram_tensor("xT16", [512, b_core], bf16, kind="ExternalInput")
    xT8_h = nc.dram_tensor("xT8", [DIM, b_core], f8, kind="ExternalInput")
    xn8_h = nc.dram_tensor("xn8", [b_core, DIM], f8, kind="ExternalInput")
    wk8_h = nc.dram_tensor("wk8", [512, DIM], f8, kind="ExternalInput")
    wk16_h = nc.dram_tensor("wk16", [512, DIM], bf16, kind="ExternalInput")
    wq8_h = nc.dram_tensor("wq8", [DIM, DIM], f8, kind="ExternalInput")
    wv16_h = nc.dram_tensor("wv16", [DIM, DIM], bf16, kind="ExternalInput")
    wlr8_h = nc.dram_tensor("wlr8", [512], f8, kind="ExternalInput")
    wlr16_h = nc.dram_tensor("wlr16", [512], bf16, kind="ExternalInput")
    bk_h = nc.dram_tensor("bk", [DIM], f32, kind="ExternalInput")     # bk * SKP
    bqs_h = nc.dram_tensor("bqs", [DIM], f32, kind="ExternalInput")   # bq - SHIFT
    blr_h = nc.dram_tensor("blr", [1], f32, kind="ExternalInput")
    # (bv - wfb) * SAR / (b_total * SG): outer-product operand for the drain
    bvcp_h = nc.dram_tensor("bvcp", [DIM], f32, kind="ExternalInput")
    wfb_h = nc.dram_tensor("wfb", [DIM], f32, kind="ExternalInput")
    out_h = nc.dram_tensor("out", [b_core, DIM], bf16, kind="ExternalOutput")

    pd_drain_scale = SAR / (float(b_core * n_cores) * SX * SG)
    fin_recip_scale = 1.0 / SAR
    q_act_scale = 1.0 / (SX * SWQ)
    k_act_scale = 1.0 / SKP
    lr_act_scale = 1.0 / (SX * SWL)

    with tile.TileContext(nc) as tc:
        with (
            tc.tile_pool(name="persist", bufs=1) as persist,
            tc.tile_pool(name="psum", bufs=6, space="PSUM") as psum,
            tc.tile_pool(name="psmall", bufs=2, space="PSUM") as psmall,
            tc.tile_pool(name="tmp", bufs=4) as tmp,
            tc.tile_pool(name="ost", bufs=6) as ost,
            tc.tile_pool(name="arst", bufs=4) as arst,
            tc.tile_pool(name="dram", bufs=1, space="DRAM") as dram,
        ):
            # ---- persistent SBUF tensors ----
            wq8 = persist.tile([P, NT, DIM], f8, name="wq8")
            xT8a = persist.tile([P, NT, b_core // 2], f8, name="xT8a")
            xT8b = persist.tile([P, NT, b_core // 2], f8, name="xT8b")
            wk8 = persist.tile([P, 4, DIM], f8, name="wk8")
            wk16 = persist.tile([P, 4, DIM], bf16, name="wk16")
            wlr8 = persist.tile([P, 4, 1], f8, name="wlr8")
            wlr16 = persist.tile([P, 4, 1], bf16, name="wlr16")
            xT16 = persist.tile([P, 4, b_core], bf16, name="xT16")
            xn8 = persist.tile([P, nbt, DIM], f8, name="xn8")
            wv16 = persist.tile([P, NT, DIM], bf16, name="wv16")
            g8 = persist.tile([P, nbt, DIM], f8, name="g8")
            et8 = persist.tile([P, NT, b_core], f8, name="et8")
            mb = persist.tile([P, NT, DIM], bf16, name="mb")
            wn8a = persist.tile([P, NT, 512], f8, name="wn8a")
            wn8b = persist.tile([P, NT, 512], f8, name="wn8b")
            bk_b = persist.tile([P, DIM], f32, name="bk_b")
            wfb_b = persist.tile([P, DIM], f32, name="wfb_b")
            bvcp_b = persist.tile([P, DIM], f32, name="bvcp_b")
            bq_c = persist.tile([P, NT], f32, name="bq_c")
            blr_c = persist.tile([P, 1], f32, name="blr_c")
            lr_c = persist.tile([P, nbt], f32, name="lr_c")
            r_c = persist.tile([P, NT], f32, name="r_c")
            recip_c = persist.tile([P, nbt], f32, name="recip_c")
            lrT_sb = persist.tile([1, b_core], f32, name="lrT_sb")
            rT_sb = persist.tile([1, DIM], f32, name="rT_sb")
            prsT_sb = persist.tile([1, b_core], f32, name="prsT_sb")
            ones8 = persist.tile([P, 2, 16], f8, name="ones8")
            ones_row = persist.tile([1, P], f32, name="ones_row")

            # ---- DRAM: AllReduce bounce (column halves) + transpose scratch ----
            ar_inA = dram.tile([DIM, 512], f8, name="ar_inA")
            ar_inB = dram.tile([DIM, 512], f8, name="ar_inB")
            ar_outA = dram.tile([DIM, 512], f8, name="ar_outA", addr_space="Shared")
            ar_outB = dram.tile([DIM, 512], f8, name="ar_outB", addr_space="Shared")
            sc_lr = dram.tile([nbt, P], f32, name="sc_lr")
            sc_r = dram.tile([NT, P], f32, name="sc_r")
            sc_prs = dram.tile([nbt, P], f32, name="sc_prs")
            warm_in = dram.tile([1, 64], f8, name="warm_in")
            warm_out = dram.tile([1, 64], f8, name="warm_out", addr_space="Shared")

            nc.vector.memset(ones8[:], 1.0)
            nc.vector.memset(ones_row[:], 1.0)

            # warm up the collective stream: the first collective pays a
            # ~50us barrier/setup cost; burn it on a tiny AllReduce that
            # runs concurrently with the startup DMAs.
            wtmp = tmp.tile([1, 64], f8, tag="kv", name="wtmp")
            nc.vector.memset(wtmp[:], 0.0)
            nc.gpsimd.dma_start(out=warm_in[0:1, :], in_=wtmp[:])
            nc.gpsimd.collective_compute(
                "AllReduce",
                mybir.AluOpType.add,
                replica_groups=[list(range(n_cores))],
                ins=[warm_in[:, :]],
                outs=[warm_out[:, :]],
            )

            # ---- small DMAs (gpsimd queue) ----
            nc.gpsimd.dma_start(
                out=bq_c[:],
                in_=bass.AP(tensor=bqs_h, offset=0, ap=[[1, P], [P, NT]]),
            )
            nc.gpsimd.dma_start(
                out=blr_c[:],
                in_=bass.AP(tensor=blr_h, offset=0, ap=[[0, P], [1, 1]]),
            )
            for i in range(4):
                nc.gpsimd.dma_start(
                    out=wlr8[:, i, :],
                    in_=bass.AP(tensor=wlr8_h, offset=i * P, ap=[[1, P], [P, 1]]),
                )
                nc.gpsimd.dma_start(
                    out=wlr16[:, i, :],
                    in_=bass.AP(tensor=wlr16_h, offset=i * P, ap=[[1, P], [P, 1]]),
                )
            # bias broadcasts across partitions via K=1 ones-matmuls
            for bi, (bias_dst, bias_src) in enumerate(
                ((bk_b, bk_h), (wfb_b, wfb_h), (bvcp_b, bvcp_h))
            ):
                for c in range(2):
                    brow = tmp.tile([1, 512], f32, tag="kv", name=f"br{bi}_{c}")
                    nc.gpsimd.dma_start(
                        out=brow[:],
                        in_=bass.AP(tensor=bias_src, offset=c * 512,
                                    ap=[[0, 1], [1, 512]]),
                    )
                    pb = psum.tile([P, 512], f32, tag="ps", name=f"pb{bi}_{c}")
                    nc.tensor.matmul(
                        pb[:], ones_row[:, :], brow[:], start=True, stop=True
                    )
                    nc.vector.tensor_copy(bias_dst[:, c * 512:(c + 1) * 512], pb[:])

            # ---- bulk DMAs: sync queue = q/k weights + xT8; scalar = rest ----
            for i in range(NT):
                nc.sync.dma_start(out=wq8[:, i, :], in_=wq8_h[i * P:(i + 1) * P, :])
            for i in range(NT):
                nc.sync.dma_start(
                    out=xT8a[:, i, 0:512], in_=xT8_h[i * P:(i + 1) * P, 0:512]
                )
            for i in range(4):
                nc.sync.dma_start(out=wk8[:, i, :], in_=wk8_h[i * P:(i + 1) * P, :])
            for i in range(4):
                nc.sync.dma_start(out=wk16[:, i, :], in_=wk16_h[i * P:(i + 1) * P, :])
            if b_core // 2 > 512:
                for i in range(NT):
                    nc.sync.dma_start(
                        out=xT8a[:, i, 512:],
                        in_=xT8_h[i * P:(i + 1) * P, 512:b_core // 2],
                    )
            for i in range(NT):
                nc.sync.dma_start(
                    out=xT8b[:, i, :], in_=xT8_h[i * P:(i + 1) * P, b_core // 2:]
                )
            # scalar queue: xT16 b-chunked (earliest b first), then xn8, wv16
            for c in range(nbc):
                for i in range(4):
                    nc.scalar.dma_start(
                        out=xT16[:, i, c * 512:(c + 1) * 512],
                        in_=xT16_h[i * P:(i + 1) * P, c * 512:(c + 1) * 512],
                    )
            for t in range(nbt):
                nc.scalar.dma_start(out=xn8[:, t, :], in_=xn8_h[t * P:(t + 1) * P, :])
            for i in range(NT):
                nc.scalar.dma_start(out=wv16[:, i, :], in_=wv16_h[i * P:(i + 1) * P, :])

            def xt8_lhs(t, j2):
                """fp8 xT lhsT pair slice for global b-tile t, i-pair j2."""
                src = xT8a if t < nbt // 2 else xT8b
                tc_ = t % (nbt // 2)
                return src[:, 2 * j2:2 * j2 + 2, tc_ * P:(tc_ + 1) * P]

            def emit_q(chunks):
                """et8 = exp(qT + bq - SHIFT), transposed layout [h, b]. fp8 DR."""
                for bc in chunks:
                    src = xT8a if bc < nbc // 2 else xT8b
                    lo = (bc % (nbc // 2)) * 512
                    for hb in range(NT):
                        pq = psum.tile([P, 512], f32, tag="ps", name=f"pq{bc}_{hb}")
                        for j in range(NT // 2):
                            nc.tensor.matmul(
                                pq[:],
                                wq8[:, 2 * j:2 * j + 2, hb * P:(hb + 1) * P],
                                src[:, 2 * j:2 * j + 2, lo:lo + 512],
                                start=(j == 0), stop=(j == NT // 2 - 1),
                                perf_mode=DR,
                            )
                        nc.scalar.activation(
                            et8[:, hb, bc * 512:(bc + 1) * 512], pq[:], AF.Exp,
                            bias=bq_c[:, hb:hb + 1], scale=q_act_scale,
                        )

            def emit_lrT(bc):
                """lr_c[p, 4bc+j] = sigmoid(x @ wlr + blr) * SG for one
                512-col chunk, via a transposed [1, 512] matmul + bounce."""
                src8 = xT8a if bc < nbc // 2 else xT8b
                lo = (bc % (nbc // 2)) * 512
                pl = psmall.tile([1, 512], f32, tag="pl", name=f"plr{bc}")
                for i in range(4):
                    nc.tensor.matmul(
                        pl[:],
                        wlr8[:, i, 0:1],
                        src8[:, i, lo:lo + 512],
                        start=(i == 0), stop=False,
                    )
                for i in range(4):
                    nc.tensor.matmul(
                        pl[:],
                        wlr16[:, i, 0:1],
                        xT16[:, i, bc * 512:(bc + 1) * 512],
                        start=False, stop=(i == 3),
                    )
                nc.scalar.activation(
                    lrT_sb[0:1, bc * 512:(bc + 1) * 512], pl[:], AF.Sigmoid,
                    bias=blr_c[0:1, 0:1], scale=lr_act_scale,
                )
                nc.gpsimd.dma_start(
                    out=sc_lr[nct * bc:nct * (bc + 1), :],
                    in_=lrT_sb[0:1, bc * 512:(bc + 1) * 512],
                )
                nc.gpsimd.dma_start(
                    out=lr_c[:, nct * bc:nct * (bc + 1)],
                    in_=sc_lr[nct * bc:nct * (bc + 1), :].rearrange("a b -> b a"),
                )
                nc.vector.tensor_scalar_mul(
                    lr_c[:, nct * bc:nct * (bc + 1)],
                    lr_c[:, nct * bc:nct * (bc + 1)], SG,
                )

            def emit_k(tiles):
                """g8 = lr * sigmoid(k) * SG, natural layout [b, h].
                Contraction split: i<512 fp8-DR, i>=512 bf16."""
                for t in tiles:
                    for c in range(2):
                        pk = psum.tile([P, 512], f32, tag="ps", name=f"pk{t}_{c}")
                        for j2 in range(2):
                            nc.tensor.matmul(
                                pk[:],
                                xt8_lhs(t, j2),
                                wk8[:, 2 * j2:2 * j2 + 2, c * 512:(c + 1) * 512],
                                start=(j2 == 0), stop=False,
                                perf_mode=DR,
                            )
                        for i in range(4):
                            nc.tensor.matmul(
                                pk[:],
                                xT16[:, i, t * P:(t + 1) * P],
                                wk16[:, i, c * 512:(c + 1) * 512],
                                start=False, stop=(i == 3),
                            )
                        ktmp = tmp.tile([P, 512], f32, tag="kv", name=f"kt{t}_{c}")
                        nc.vector.tensor_add(
                            ktmp[:], pk[:], bk_b[:, c * 512:(c + 1) * 512]
                        )
                        sgk = tmp.tile([P, 512], bf16, tag="sg", name=f"sg{t}_{c}")
                        nc.scalar.activation(sgk[:], ktmp[:], AF.Sigmoid,
                                             scale=k_act_scale)
                        nc.scalar.activation(
                            g8[:, t, c * 512:(c + 1) * 512], sgk[:], AF.Copy,
                            scale=lr_c[:, t:t + 1],
                        )

            def emit_m():
                """mb = x.T @ g (per-core partial), [i, h] layout, fp8 DR;
                then rT = ones.T @ g via [1, 512] matmuls + bounce."""
                for hc in range(2):
                    for ib in range(NT):
                        pm = psum.tile([P, 512], f32, tag="ps", name=f"pm{hc}_{ib}")
                        for bp in range(nbt // 2):
                            nc.tensor.matmul(
                                pm[:],
                                xn8[:, 2 * bp:2 * bp + 2, ib * P:(ib + 1) * P],
                                g8[:, 2 * bp:2 * bp + 2, hc * 512:(hc + 1) * 512],
                                start=(bp == 0), stop=(bp == nbt // 2 - 1),
                                perf_mode=DR,
                            )
                        nc.vector.tensor_copy(
                            mb[:, ib, hc * 512:(hc + 1) * 512], pm[:]
                        )
                for hc in range(2):
                    pr = psmall.tile([1, 512], f32, tag="pl", name=f"pr{hc}")
                    for bt in range(nbt):
                        nc.tensor.matmul(
                            pr[:],
                            ones8[:, 0, 0:1],
                            g8[:, bt, hc * 512:(hc + 1) * 512],
                            start=(bt == 0), stop=(bt == nbt - 1),
                        )
                    nc.vector.tensor_copy(
                        rT_sb[0:1, hc * 512:(hc + 1) * 512], pr[:]
                    )
                    nc.gpsimd.dma_start(
                        out=sc_r[nct * hc:nct * (hc + 1), :],
                        in_=rT_sb[0:1, hc * 512:(hc + 1) * 512],
                    )
                nc.gpsimd.dma_start(
                    out=r_c[:, :], in_=sc_r[:, :].rearrange("a b -> b a")
                )

            def emit_pd(oc):
                """delta.T partial [:, oc half] = mb.T @ wv + r x bvc,
                drained fp8 to one AR column-half."""
                for hb in range(NT):
                    pd = psum.tile([P, 512], f32, tag="ps", name=f"pd{hb}_{oc}")
                    for i in range(NT):
                        nc.tensor.matmul(
                            pd[:],
                            mb[:, i, hb * P:(hb + 1) * P],
                            wv16[:, i, oc * 512:(oc + 1) * 512],
                            start=(i == 0), stop=(i == NT - 1),
                        )
                    pt = tmp.tile([P, 512], f32, tag="kv", name=f"pt{hb}_{oc}")
                    nc.scalar.activation(
                        pt[:], pd[:], AF.Copy, scale=pd_drain_scale
                    )
                    dst = arst.tile([P, 512], f8, tag="ar", name=f"ds{hb}_{oc}")
                    nc.vector.scalar_tensor_tensor(
                        dst[:],
                        bvcp_b[:, oc * 512:(oc + 1) * 512],
                        r_c[:, hb:hb + 1],
                        pt[:],
                        op0=ALU.mult,
                        op1=ALU.add,
                    )
                    ar_dst = ar_inA if oc == 0 else ar_inB
                    eng = nc.sync if oc == 0 else nc.scalar
                    eng.dma_start(
                        out=ar_dst[hb * P:(hb + 1) * P, :], in_=dst[:]
                    )

            def emit_prsT(bc):
                """prsT[b] = sum_h et8[h, b] for one 512-col chunk."""
                pp = psmall.tile([1, 512], f32, tag="pl", name=f"pp{bc}")
                for hb in range(NT):
                    nc.tensor.matmul(
                        pp[:],
                        ones8[:, 0, 0:1],
                        et8[:, hb, bc * 512:(bc + 1) * 512],
                        start=(hb == 0), stop=(hb == NT - 1),
                    )
                nc.vector.tensor_copy(prsT_sb[0:1, bc * 512:(bc + 1) * 512], pp[:])
                nc.sync.dma_start(
                    out=sc_prs[nct * bc:nct * (bc + 1), :],
                    in_=prsT_sb[0:1, bc * 512:(bc + 1) * 512],
                )

            def emit_recip():
                nc.sync.dma_start(
                    out=recip_c[:, :], in_=sc_prs[:, :].rearrange("a b -> b a")
                )
                nc.vector.reciprocal(recip_c[:], recip_c[:])
                nc.vector.tensor_scalar_mul(recip_c[:], recip_c[:], fin_recip_scale)

            def emit_fin(oc, wn):
                """out[:, oc half] = (et8.T @ wn) * recip + wfb, fp8 DR."""
                for t in range(nbt):
                    po = psum.tile([P, 512], f32, tag="ps", name=f"po{t}_{oc}")
                    for j in range(NT // 2):
                        nc.tensor.matmul(
                            po[:],
                            et8[:, 2 * j:2 * j + 2, t * P:(t + 1) * P],
                            wn[:, 2 * j:2 * j + 2, :],
                            start=(j == 0), stop=(j == NT // 2 - 1),
                            perf_mode=DR,
                        )
                    o_st = ost.tile([P, 512], bf16, tag="os", name=f"os{t}_{oc}")
                    nc.vector.scalar_tensor_tensor(
                        o_st[:],
                        po[:],
                        recip_c[:, t:t + 1],
                        wfb_b[:, oc * 512:(oc + 1) * 512],
                        op0=ALU.mult,
                        op1=ALU.add,
                    )
                    eng = nc.sync if oc == 0 else nc.scalar
                    eng.dma_start(
                        out=out_h[t * P:(t + 1) * P, oc * 512:(oc + 1) * 512],
                        in_=o_st[:],
                    )

            # ---- schedule ----
            emit_q([0])           # q chunk 0 warms up the PE
            emit_lrT(0)
            for bc in range(1, nbc):
                emit_k(range(nct * (bc - 1), nct * bc))
                emit_lrT(bc)
            emit_k(range(nct * (nbc - 1), nct * nbc))
            emit_m()
            emit_pd(0)
            nc.gpsimd.collective_compute(
                "AllReduce",
                mybir.AluOpType.add,
                replica_groups=[list(range(n_cores))],
                ins=[ar_inA[:, :]],
                outs=[ar_outA[:, :]],
            )
            emit_pd(1)
            nc.gpsimd.collective_compute(
                "AllReduce",
                mybir.AluOpType.add,
                replica_groups=[list(range(n_cores))],
                ins=[ar_inB[:, :]],
                outs=[ar_outB[:, :]],
            )
            for hb in range(NT):
                nc.scalar.dma_start(
                    out=wn8a[:, hb, :], in_=ar_outA[hb * P:(hb + 1) * P, :]
                )
            for hb in range(NT):
                nc.scalar.dma_start(
                    out=wn8b[:, hb, :], in_=ar_outB[hb * P:(hb + 1) * P, :]
                )
            emit_prsT(0)
            for bc in range(1, nbc):
                emit_q([bc])
                emit_prsT(bc)
            emit_recip()
            emit_fin(0, wn8a)
            emit_fin(1, wn8b)

    nc.compile()
    return nc


def _host_prep(x, W_slow_w, W_slow_b, W_fast_b, b_core, n_cores):
    """Shard + pre-transpose + cast inputs; returns per-core input maps."""
    Wk = W_slow_w[:DIM]
    Wv = W_slow_w[DIM:2 * DIM]
    Wq = W_slow_w[2 * DIM:3 * DIM]
    wlr = W_slow_w[3 * DIM]

    WkT = np.ascontiguousarray(Wk.T)
    wk8 = np.clip(WkT[:512, :] * SWK, -240.0, 240.0).astype(F8E4)
    wk16 = (WkT[512:, :] * SKP).astype(BF16)
    wv16 = np.ascontiguousarray(Wv.T).astype(BF16)
    wq8 = np.clip(np.ascontiguousarray(Wq.T) * SWQ, -240.0, 240.0).astype(F8E4)
    wlr8 = np.clip(wlr[:512] * SWL, -240.0, 240.0).astype(F8E4)
    wlr16 = (wlr[512:] * (SX * SWL)).astype(BF16)

    bk = (W_slow_b[:DIM] * SKP).astype(np.float32)
    b_total = float(b_core * n_cores)
    bvcp = ((W_slow_b[DIM:2 * DIM] - W_fast_b) * (SAR / (b_total * SG))).astype(
        np.float32
    )
    bqs = (W_slow_b[2 * DIM:3 * DIM] - SHIFT).astype(np.float32)
    blr = np.ascontiguousarray(W_slow_b[3 * DIM:3 * DIM + 1]).astype(np.float32)
    wfb = np.ascontiguousarray(W_fast_b).astype(np.float32)

    in_maps = []
    for c in range(n_cores):
        xs = x[c * b_core:(c + 1) * b_core, :]
        xT = np.ascontiguousarray(xs.T)
        xT16 = np.ascontiguousarray(xT[512:]).astype(BF16)
        xT8 = np.clip(xT * SX, -240.0, 240.0).astype(F8E4)
        xn8 = np.clip(xs * SX, -240.0, 240.0).astype(F8E4)
        in_maps.append({
            "xT16": xT16, "xT8": xT8, "xn8": np.ascontiguousarray(xn8),
            "wk8": wk8, "wk16": wk16, "wq8": wq8, "wv16": wv16,
            "wlr8": wlr8, "wlr16": wlr16,
            "bk": bk, "bqs": bqs, "blr": blr, "bvcp": bvcp, "wfb": wfb,
        })
    return in_maps


_PROGRAM_CACHE = {}


def _get_program(b_core, n_cores=N_CORES):
    key = (b_core, n_cores)
    if key not in _PROGRAM_CACHE:
        _PROGRAM_CACHE[key] = _build_program(b_core, n_cores)
    return _PROGRAM_CACHE[key]


def _run_device(x, W_slow_w, W_slow_b, W_fast_b, trace=False):
    from concourse.bass_utils import run_bass_kernel_spmd

    b_core = x.shape[0] // N_CORES
    nc = _get_program(b_core)
    in_maps = _host_prep(x, W_slow_w, W_slow_b, W_fast_b, b_core, N_CORES)
    res = run_bass_kernel_spmd(nc, in_maps, list(range(N_CORES)), trace=trace)
    out = np.concatenate([res.results[c]["out"] for c in range(N_CORES)], axis=0)
    return out.astype(np.float32), res


def _reference_numpy(x, W_slow_w, W_slow_b, W_fast_w, W_fast_b):
    """Exact fallback (only used if W_fast_w != 0, which the spec never produces)."""
    x = x.astype(np.float64)
    s = x @ W_slow_w.astype(np.float64).T + W_slow_b.astype(np.float64)
    k = s[:, :DIM]
    v = s[:, DIM:2 * DIM]
    q = s[:, 2 * DIM:3 * DIM]
    lr = 1.0 / (1.0 + np.exp(-s[:, -1:]))
    ek = np.exp(k - k.max(axis=1, keepdims=True))
    ak = ek / ek.sum(axis=1, keepdims=True)
    v_bar = ak @ W_fast_w.astype(np.float64).T + W_fast_b.astype(np.float64)
    sigk = 1.0 / (1.0 + np.exp(-k))
    delta = (lr * (v - v_bar)).T @ sigk / x.shape[0]
    w_new = W_fast_w.astype(np.float64) + delta
    eq = np.exp(q - q.max(axis=1, keepdims=True))
    aq = eq / eq.sum(axis=1, keepdims=True)
    return (aq @ w_new.T + W_fast_b.astype(np.float64)).astype(np.float32)


def kernel(x, W_slow_w, W_slow_b, W_fast_w, W_fast_b):
    x = np.asarray(x)
    W_slow_w = np.asarray(W_slow_w)
    W_slow_b = np.asarray(W_slow_b)
    W_fast_w = np.asarray(W_fast_w)
    W_fast_b = np.asarray(W_fast_b)
    if np.any(W_fast_w):
        # Spec guarantees W_fast_w == 0; exact fallback for generality.
        return _reference_numpy(x, W_slow_w, W_slow_b, W_fast_w, W_fast_b)
    out, _ = _run_device(x, W_slow_w, W_slow_b, W_fast_b, trace=False)
    return out


# revision 22
# speedup vs baseline: 1.3823x; 1.0230x over previous
"""DeltaNet fused kernel for 8 TRN2 NeuronCores (Bass/Tile), fp8-hybrid v2.

Math (reference, with W_fast_w == 0 so v_bar == W_fast_b):
    s  = x @ W_slow_w.T + W_slow_b            [B, 3073]
    k  = s[:, :1024]; v = s[:, 1024:2048]; q = s[:, 2048:3072]
    lr = sigmoid(s[:, 3072])
    delta[o,h] = sum_b (lr*(v - wfb))[b,o] * sigmoid(k)[b,h] / B
    out = softmax(q) @ delta.T + wfb

Restructured to eliminate the v projection (v = x @ Wv.T + bv):
    g  = lr * sigmoid(k)                      [B, H]
    M  = x.T @ g                              [I, H]   (per-core partial)
    r  = sum_b g[b, :]                        [H]
    delta.T = (M.T @ Wv.T + r x (bv - wfb)) / B        [H, O]  (AllReduced)
    out = softmax(q) @ delta.T + wfb

Precision (validated in numpy simulation; end-to-end rel err 6.6e-3 vs 2e-2
tolerance): fp8-e4m3 DoubleRow for q / M / final matmuls and the first half
of the k contraction; bf16 for the rest of k, lr, and the delta.T matmul.
The AllReduce runs in fp8 (delta scaled by 4096), split into two [H, 512]
column-halves so the final matmul's first half can start after the first AR.

Schedule: q chunk 0 warms up the PE while the k operands stream in; the
whole rest of the q-phase plus the softmax row-sums run after the AR
trigger to hide the collective; the final matmul is split into per-AR-half
passes. lr / r / rowsum are computed as transposed [1, N] matmuls (cheap
N=512 streams instead of 384 N=1 matmuls) and moved cross-partition via
tiny DRAM round-trips.
"""

import os
import sys

for _p in ("/opt/trn_rl_repo", "/root/.axon_site/_ro/trn_rl_repo"):
    if os.path.isdir(_p) and _p not in sys.path:
        sys.path.append(_p)

import numpy as np
import ml_dtypes

BF16 = ml_dtypes.bfloat16
F8E4 = ml_dtypes.float8_e4m3     # TRN fp8e4: max normal +-240

N_CORES = 8
B_FULL = 16384
DIM = 1024          # dim_in == dim_out == dim_hidden
P = 128
NT = DIM // P       # 8 tiles along any 1024 dim

SX = 16.0           # x fp8 scale             (|x|max 5.4  -> 87)
SWQ = 512.0         # Wq fp8 scale            (|w|max .16  -> 80)
SWK = 512.0         # Wk fp8 scale (i < 512 half)
SKP = SX * SWK      # k psum scale (bf16 half pre-scaled to match)
SWL = 512.0         # wlr fp8 scale (i < 512 half)
SG = 128.0          # g fp8 scale             (g in (0,1)  -> <128)
SAR = 4096.0        # delta fp8 scale         (|delta|max .043 -> 176)
SHIFT = 3.0         # exp shift               (max q+bq 7.63 -> et < 103)


def _build_program(b_core: int, n_cores: int = N_CORES):
    """Build the SPMD Bass program (same program on every core)."""
    import concourse.bass as bass
    import concourse.mybir as mybir
    import concourse.tile as tile
    from concourse import bacc

    f32 = mybir.dt.float32
    bf16 = mybir.dt.bfloat16
    f8 = mybir.dt.float8e4
    AF = mybir.ActivationFunctionType
    ALU = mybir.AluOpType
    DR = mybir.MatmulPerfMode.DoubleRow

    nbt = b_core // P               # b-tiles per core (16)
    nbc = b_core // 512             # 512-wide b-chunks (4)
    nct = 512 // P                  # b-tiles per chunk (4)
    assert b_core % 1024 == 0

    nc = bacc.Bacc(
        "TRN2",
        target_bir_lowering=False,
        debug=False,
        num_devices=n_cores,
    )

    # ---- kernel I/O ----
    xT16_h = nc.dram_tensor("xT16", [512, b_core], bf16, kind="ExternalInput")
    xT8_h = nc.dram_tensor("xT8", [DIM, b_core], f8, kind="ExternalInput")
    xn8_h = nc.dram_tensor("xn8", [b_core, DIM], f8, kind="ExternalInput")
    wk8_h = nc.dram_tensor("wk8", [512, DIM], f8, kind="ExternalInput")
    wk16_h = nc.dram_tensor("wk16", [512, DIM], bf16, kind="ExternalInput")
    wq8_h = nc.dram_tensor("wq8", [DIM, DIM], f8, kind="ExternalInput")
    wv16_h = nc.dram_tensor("wv16", [DIM, DIM], bf16, kind="ExternalInput")
    wlr8_h = nc.dram_tensor("wlr8", [512], f8, kind="ExternalInput")
    wlr16_h = nc.dram_tensor("wlr16", [512], bf16, kind="ExternalInput")
    bk_h = nc.dram_tensor("bk", [DIM], f32, kind="ExternalInput")     # bk * SKP
    bqs_h = nc.dram_tensor("bqs", [DIM], f32, kind="ExternalInput")   # bq - SHIFT
    blr_h = nc.dram_tensor("blr", [1], f32, kind="ExternalInput")
    # (bv - wfb) * SAR / (b_total * SG): outer-product operand for the drain
    bvcp_h = nc.dram_tensor("bvcp", [DIM], f32, kind="ExternalInput")
    wfb_h = nc.dram_tensor("wfb", [DIM], f32, kind="ExternalInput")
    out_h = nc.dram_tensor("out", [b_core, DIM], bf16, kind="ExternalOutput")

    pd_drain_scale = SAR / (float(b_core * n_cores) * SX * SG)
    fin_recip_scale = 1.0 / SAR
    q_act_scale = 1.0 / (SX * SWQ)
    k_act_scale = 1.0 / SKP
    lr_act_scale = 1.0 / (SX * SWL)

    with tile.TileContext(nc) as tc:
        with (
            tc.tile_pool(name="persist", bufs=1) as persist,
            tc.tile_pool(name="psum", bufs=6, space="PSUM") as psum,
            tc.tile_pool(name="psmall", bufs=2, space="PSUM") as psmall,
            tc.tile_pool(name="tmp", bufs=4) as tmp,
            tc.tile_pool(name="ost", bufs=6) as ost,
            tc.tile_pool(name="arst", bufs=4) as arst,
            tc.tile_pool(name="dram", bufs=1, space="DRAM") as dram,
        ):
            # ---- persistent SBUF tensors ----
            wq8 = persist.tile([P, NT, DIM], f8, name="wq8")
            xT8a = persist.tile([P, NT, b_core // 2], f8, name="xT8a")
            xT8b = persist.tile([P, NT, b_core // 2], f8, name="xT8b")
            wk8 = persist.tile([P, 4, DIM], f8, name="wk8")
            wk16 = persist.tile([P, 4, DIM], bf16, name="wk16")
            wlr8 = persist.tile([P, 4, 1], f8, name="wlr8")
            wlr16 = persist.tile([P, 4, 1], bf16, name="wlr16")
            xT16 = persist.tile([P, 4, b_core], bf16, name="xT16")
            xn8 = persist.tile([P, nbt, DIM], f8, name="xn8")
            wv16 = persist.tile([P, NT, DIM], bf16, name="wv16")
            g8 = persist.tile([P, nbt, DIM], f8, name="g8")
            et8 = persist.tile([P, NT, b_core], f8, name="et8")
            mb = persist.tile([P, NT, DIM], bf16, name="mb")
            wn8a = persist.tile([P, NT, 512], f8, name="wn8a")
            wn8b = persist.tile([P, NT, 512], f8, name="wn8b")
            bk_b = persist.tile([P, DIM], f32, name="bk_b")
            wfb_b = persist.tile([P, DIM], f32, name="wfb_b")
            bvcp_b = persist.tile([P, DIM], f32, name="bvcp_b")
            bq_c = persist.tile([P, NT], f32, name="bq_c")
            blr_c = persist.tile([P, 1], f32, name="blr_c")
            lr_c = persist.tile([P, nbt], f32, name="lr_c")
            r_c = persist.tile([P, NT], f32, name="r_c")
            recip_c = persist.tile([P, nbt], f32, name="recip_c")
            lrT_sb = persist.tile([1, b_core], f32, name="lrT_sb")
            rT_sb = persist.tile([1, DIM], f32, name="rT_sb")
            prsT_sb = persist.tile([1, b_core], f32, name="prsT_sb")
            ones8 = persist.tile([P, 2, 16], f8, name="ones8")
            ones_row = persist.tile([1, P], f32, name="ones_row")

            # ---- DRAM: AllReduce bounce (column halves) + transpose scratch ----
            ar_inA = dram.tile([DIM, 512], f8, name="ar_inA")
            ar_inB = dram.tile([DIM, 512], f8, name="ar_inB")
            ar_outA = dram.tile([DIM, 512], f8, name="ar_outA", addr_space="Shared")
            ar_outB = dram.tile([DIM, 512], f8, name="ar_outB", addr_space="Shared")
            sc_lr = dram.tile([nbt, P], f32, name="sc_lr")
            sc_r = dram.tile([NT, P], f32, name="sc_r")
            sc_prs = dram.tile([nbt, P], f32, name="sc_prs")
            warm_in = dram.tile([1, 64], f8, name="warm_in")
            warm_out = dram.tile([1, 64], f8, name="warm_out", addr_space="Shared")

            nc.vector.memset(ones8[:], 1.0)
            nc.vector.memset(ones_row[:], 1.0)

            # warm up the collective stream: the first collective pays a
            # ~50us barrier/setup cost; burn it on a tiny AllReduce that
            # runs concurrently with the startup DMAs.
            wtmp = tmp.tile([1, 64], f8, tag="kv", name="wtmp")
            nc.vector.memset(wtmp[:], 0.0)
            nc.gpsimd.dma_start(out=warm_in[0:1, :], in_=wtmp[:])
            nc.gpsimd.collective_compute(
                "AllReduce",
                mybir.AluOpType.add,
                replica_groups=[list(range(n_cores))],
                ins=[warm_in[:, :]],
                outs=[warm_out[:, :]],
            )

            # ---- small DMAs (gpsimd queue) ----
            nc.gpsimd.dma_start(
                out=bq_c[:],
                in_=bass.AP(tensor=bqs_h, offset=0, ap=[[1, P], [P, NT]]),
            )
            nc.gpsimd.dma_start(
                out=blr_c[:],
                in_=bass.AP(tensor=blr_h, offset=0, ap=[[0, P], [1, 1]]),
            )
            for i in range(4):
                nc.gpsimd.dma_start(
                    out=wlr8[:, i, :],
                    in_=bass.AP(tensor=wlr8_h, offset=i * P, ap=[[1, P], [P, 1]]),
                )
                nc.gpsimd.dma_start(
                    out=wlr16[:, i, :],
                    in_=bass.AP(tensor=wlr16_h, offset=i * P, ap=[[1, P], [P, 1]]),
                )
            # bias broadcasts across partitions via K=1 ones-matmuls
            for bi, (bias_dst, bias_src) in enumerate(
                ((bk_b, bk_h), (wfb_b, wfb_h), (bvcp_b, bvcp_h))
            ):
                for c in range(2):
                    brow = tmp.tile([1, 512], f32, tag="kv", name=f"br{bi}_{c}")
                    nc.scalar.dma_start(
                        out=brow[:],
                        in_=bass.AP(tensor=bias_src, offset=c * 512,
                                    ap=[[0, 1], [1, 512]]),
                    )
                    pb = psum.tile([P, 512], f32, tag="ps", name=f"pb{bi}_{c}")
                    nc.tensor.matmul(
                        pb[:], ones_row[:, :], brow[:], start=True, stop=True
                    )
                    nc.vector.tensor_copy(bias_dst[:, c * 512:(c + 1) * 512], pb[:])

            # ---- bulk DMAs: sync queue = q/k weights + xT8; scalar = rest ----
            for i in range(NT):
                nc.gpsimd.dma_start(out=wq8[:, i, :], in_=wq8_h[i * P:(i + 1) * P, :])
            for i in range(NT):
                nc.sync.dma_start(
                    out=xT8a[:, i, 0:512], in_=xT8_h[i * P:(i + 1) * P, 0:512]
                )
            for i in range(4):
                nc.sync.dma_start(out=wk8[:, i, :], in_=wk8_h[i * P:(i + 1) * P, :])
            for i in range(4):
                nc.sync.dma_start(out=wk16[:, i, :], in_=wk16_h[i * P:(i + 1) * P, :])
            if b_core // 2 > 512:
                for i in range(NT):
                    nc.sync.dma_start(
                        out=xT8a[:, i, 512:],
                        in_=xT8_h[i * P:(i + 1) * P, 512:b_core // 2],
                    )
            for i in range(NT):
                nc.sync.dma_start(
                    out=xT8b[:, i, :], in_=xT8_h[i * P:(i + 1) * P, b_core // 2:]
                )
            # scalar queue: xT16 b-chunked (earliest b first), then xn8, wv16
            for c in range(nbc):
                for i in range(4):
                    nc.scalar.dma_start(
                        out=xT16[:, i, c * 512:(c + 1) * 512],
                        in_=xT16_h[i * P:(i + 1) * P, c * 512:(c + 1) * 512],
                    )
            for t in range(nbt):
                nc.scalar.dma_start(out=xn8[:, t, :], in_=xn8_h[t * P:(t + 1) * P, :])
            for i in range(NT):
                nc.scalar.dma_start(out=wv16[:, i, :], in_=wv16_h[i * P:(i + 1) * P, :])

            def xt8_lhs(t, j2):
                """fp8 xT lhsT pair slice for global b-tile t, i-pair j2."""
                src = xT8a if t < nbt // 2 else xT8b
                tc_ = t % (nbt // 2)
                return src[:, 2 * j2:2 * j2 + 2, tc_ * P:(tc_ + 1) * P]

            def emit_q(chunks):
                """et8 = exp(qT + bq - SHIFT), transposed layout [h, b]. fp8 DR."""
                for bc in chunks:
                    src = xT8a if bc < nbc // 2 else xT8b
                    lo = (bc % (nbc // 2)) * 512
                    for hb in range(NT):
                        pq = psum.tile([P, 512], f32, tag="ps", name=f"pq{bc}_{hb}")
                        for j in range(NT // 2):
                            nc.tensor.matmul(
                                pq[:],
                                wq8[:, 2 * j:2 * j + 2, hb * P:(hb + 1) * P],
                                src[:, 2 * j:2 * j + 2, lo:lo + 512],
                                start=(j == 0), stop=(j == NT // 2 - 1),
                                perf_mode=DR,
                            )
                        nc.scalar.activation(
                            et8[:, hb, bc * 512:(bc + 1) * 512], pq[:], AF.Exp,
                            bias=bq_c[:, hb:hb + 1], scale=q_act_scale,
                        )

            def emit_lrT(bc):
                """lr_c[p, 4bc+j] = sigmoid(x @ wlr + blr) * SG for one
                512-col chunk, via a transposed [1, 512] matmul + bounce."""
                src8 = xT8a if bc < nbc // 2 else xT8b
                lo = (bc % (nbc // 2)) * 512
                pl = psmall.tile([1, 512], f32, tag="pl", name=f"plr{bc}")
                for i in range(4):
                    nc.tensor.matmul(
                        pl[:],
                        wlr8[:, i, 0:1],
                        src8[:, i, lo:lo + 512],
                        start=(i == 0), stop=False,
                    )
                for i in range(4):
                    nc.tensor.matmul(
                        pl[:],
                        wlr16[:, i, 0:1],
                        xT16[:, i, bc * 512:(bc + 1) * 512],
                        start=False, stop=(i == 3),
                    )
                nc.scalar.activation(
                    lrT_sb[0:1, bc * 512:(bc + 1) * 512], pl[:], AF.Sigmoid,
                    bias=blr_c[0:1, 0:1], scale=lr_act_scale,
                )
                nc.gpsimd.dma_start(
                    out=sc_lr[nct * bc:nct * (bc + 1), :],
                    in_=lrT_sb[0:1, bc * 512:(bc + 1) * 512],
                )
                nc.gpsimd.dma_start(
                    out=lr_c[:, nct * bc:nct * (bc + 1)],
                    in_=sc_lr[nct * bc:nct * (bc + 1), :].rearrange("a b -> b a"),
                )
                nc.vector.tensor_scalar_mul(
                    lr_c[:, nct * bc:nct * (bc + 1)],
                    lr_c[:, nct * bc:nct * (bc + 1)], SG,
                )

            def emit_k(tiles):
                """g8 = lr * sigmoid(k) * SG, natural layout [b, h].
                Contraction split: i<512 fp8-DR, i>=512 bf16."""
                for t in tiles:
                    for c in range(2):
                        pk = psum.tile([P, 512], f32, tag="ps", name=f"pk{t}_{c}")
                        for j2 in range(2):
                            nc.tensor.matmul(
                                pk[:],
                                xt8_lhs(t, j2),
                                wk8[:, 2 * j2:2 * j2 + 2, c * 512:(c + 1) * 512],
                                start=(j2 == 0), stop=False,
                                perf_mode=DR,
                            )
                        for i in range(4):
                            nc.tensor.matmul(
                                pk[:],
                                xT16[:, i, t * P:(t + 1) * P],
                                wk16[:, i, c * 512:(c + 1) * 512],
                                start=False, stop=(i == 3),
                            )
                        ktmp = tmp.tile([P, 512], f32, tag="kv", name=f"kt{t}_{c}")
                        nc.vector.tensor_add(
                            ktmp[:], pk[:], bk_b[:, c * 512:(c + 1) * 512]
                        )
                        sgk = tmp.tile([P, 512], bf16, tag="sg", name=f"sg{t}_{c}")
                        nc.scalar.activation(sgk[:], ktmp[:], AF.Sigmoid,
                                             scale=k_act_scale)
                        nc.scalar.activation(
                            g8[:, t, c * 512:(c + 1) * 512], sgk[:], AF.Copy,
                            scale=lr_c[:, t:t + 1],
                        )

            def emit_m():
                """mb = x.T @ g (per-core partial), [i, h] layout, fp8 DR;
                then rT = ones.T @ g via [1, 512] matmuls + bounce."""
                for hc in range(2):
                    for ib in range(NT):
                        pm = psum.tile([P, 512], f32, tag="ps", name=f"pm{hc}_{ib}")
                        for bp in range(nbt // 2):
                            nc.tensor.matmul(
                                pm[:],
                                xn8[:, 2 * bp:2 * bp + 2, ib * P:(ib + 1) * P],
                                g8[:, 2 * bp:2 * bp + 2, hc * 512:(hc + 1) * 512],
                                start=(bp == 0), stop=(bp == nbt // 2 - 1),
                                perf_mode=DR,
                            )
                        nc.vector.tensor_copy(
                            mb[:, ib, hc * 512:(hc + 1) * 512], pm[:]
                        )
                for hc in range(2):
                    pr = psmall.tile([1, 512], f32, tag="pl", name=f"pr{hc}")
                    for bp in range(nbt // 2):
                        nc.tensor.matmul(
                            pr[:],
                            ones8[:, 0:2, 0:1],
                            g8[:, 2 * bp:2 * bp + 2, hc * 512:(hc + 1) * 512],
                            start=(bp == 0), stop=(bp == nbt // 2 - 1),
                            perf_mode=DR,
                        )
                    nc.vector.tensor_copy(
                        rT_sb[0:1, hc * 512:(hc + 1) * 512], pr[:]
                    )
                    nc.gpsimd.dma_start(
                        out=sc_r[nct * hc:nct * (hc + 1), :],
                        in_=rT_sb[0:1, hc * 512:(hc + 1) * 512],
                    )
                nc.gpsimd.dma_start(
                    out=r_c[:, :], in_=sc_r[:, :].rearrange("a b -> b a")
                )

            def emit_pd(oc):
                """delta.T partial [:, oc half] = mb.T @ wv + r x bvc,
                drained fp8 to one AR column-half."""
                for hb in range(NT):
                    pd = psum.tile([P, 512], f32, tag="ps", name=f"pd{hb}_{oc}")
                    for i in range(NT):
                        nc.tensor.matmul(
                            pd[:],
                            mb[:, i, hb * P:(hb + 1) * P],
                            wv16[:, i, oc * 512:(oc + 1) * 512],
                            start=(i == 0), stop=(i == NT - 1),
                        )
                    pt = tmp.tile([P, 512], f32, tag="kv", name=f"pt{hb}_{oc}")
                    nc.scalar.activation(
                        pt[:], pd[:], AF.Copy, scale=pd_drain_scale
                    )
                    dst = arst.tile([P, 512], f8, tag="ar", name=f"ds{hb}_{oc}")
                    nc.vector.scalar_tensor_tensor(
                        dst[:],
                        bvcp_b[:, oc * 512:(oc + 1) * 512],
                        r_c[:, hb:hb + 1],
                        pt[:],
                        op0=ALU.mult,
                        op1=ALU.add,
                    )
                    ar_dst = ar_inA if oc == 0 else ar_inB
                    eng = nc.sync if oc == 0 else nc.scalar
                    eng.dma_start(
                        out=ar_dst[hb * P:(hb + 1) * P, :], in_=dst[:]
                    )

            def emit_prsT(bc):
                """prsT[b] = sum_h et8[h, b] for one 512-col chunk."""
                pp = psmall.tile([1, 512], f32, tag="pl", name=f"pp{bc}")
                for j in range(NT // 2):
                    nc.tensor.matmul(
                        pp[:],
                        ones8[:, 0:2, 0:1],
                        et8[:, 2 * j:2 * j + 2, bc * 512:(bc + 1) * 512],
                        start=(j == 0), stop=(j == NT // 2 - 1),
                        perf_mode=DR,
                    )
                nc.vector.tensor_copy(prsT_sb[0:1, bc * 512:(bc + 1) * 512], pp[:])
                nc.sync.dma_start(
                    out=sc_prs[nct * bc:nct * (bc + 1), :],
                    in_=prsT_sb[0:1, bc * 512:(bc + 1) * 512],
                )

            def emit_recip():
                nc.sync.dma_start(
                    out=recip_c[:, :], in_=sc_prs[:, :].rearrange("a b -> b a")
                )
                nc.vector.reciprocal(recip_c[:], recip_c[:])
                nc.vector.tensor_scalar_mul(recip_c[:], recip_c[:], fin_recip_scale)

            def emit_fin(oc, wn):
                """out[:, oc half] = (et8.T @ wn) * recip + wfb, fp8 DR."""
                for t in range(nbt):
                    po = psum.tile([P, 512], f32, tag="ps", name=f"po{t}_{oc}")
                    for j in range(NT // 2):
                        nc.tensor.matmul(
                            po[:],
                            et8[:, 2 * j:2 * j + 2, t * P:(t + 1) * P],
                            wn[:, 2 * j:2 * j + 2, :],
                            start=(j == 0), stop=(j == NT // 2 - 1),
                            perf_mode=DR,
                        )
                    o_st = ost.tile([P, 512], bf16, tag="os", name=f"os{t}_{oc}")
                    nc.vector.scalar_tensor_tensor(
                        o_st[:],
                        po[:],
                        recip_c[:, t:t + 1],
                        wfb_b[:, oc * 512:(oc + 1) * 512],
                        op0=ALU.mult,
                        op1=ALU.add,
                    )
                    eng = nc.sync if oc == 0 else nc.scalar
                    eng.dma_start(
                        out=out_h[t * P:(t + 1) * P, oc * 512:(oc + 1) * 512],
                        in_=o_st[:],
                    )

            # ---- schedule ----
            emit_q([0])           # q chunk 0 warms up the PE
            emit_lrT(0)
            for bc in range(1, nbc):
                emit_k(range(nct * (bc - 1), nct * bc))
                emit_lrT(bc)
            emit_k(range(nct * (nbc - 1), nct * nbc))
            emit_m()
            emit_pd(0)
            nc.gpsimd.collective_compute(
                "AllReduce",
                mybir.AluOpType.add,
                replica_groups=[list(range(n_cores))],
                ins=[ar_inA[:, :]],
                outs=[ar_outA[:, :]],
            )
            emit_pd(1)
            nc.gpsimd.collective_compute(
                "AllReduce",
                mybir.AluOpType.add,
                replica_groups=[list(range(n_cores))],
                ins=[ar_inB[:, :]],
                outs=[ar_outB[:, :]],
            )
            for hb in range(NT):
                nc.scalar.dma_start(
                    out=wn8a[:, hb, :], in_=ar_outA[hb * P:(hb + 1) * P, :]
                )
            for hb in range(NT):
                nc.scalar.dma_start(
                    out=wn8b[:, hb, :], in_=ar_outB[hb * P:(hb + 1) * P, :]
                )
            emit_prsT(0)
            for bc in range(1, nbc):
                emit_q([bc])
                emit_prsT(bc)
            emit_recip()
            emit_fin(0, wn8a)
            emit_fin(1, wn8b)

    nc.compile()
    return nc


def _host_prep(x, W_slow_w, W_slow_b, W_fast_b, b_core, n_cores):
    """Shard + pre-transpose + cast inputs; returns per-core input maps."""
    Wk = W_slow_w[:DIM]
    Wv = W_slow_w[DIM:2 * DIM]
    Wq = W_slow_w[2 * DIM:3 * DIM]
    wlr = W_slow_w[3 * DIM]

    WkT = np.ascontiguousarray(Wk.T)
    wk8 = np.clip(WkT[:512, :] * SWK, -240.0, 240.0).astype(F8E4)
    wk16 = (WkT[512:, :] * SKP).astype(BF16)
    wv16 = np.ascontiguousarray(Wv.T).astype(BF16)
    wq8 = np.clip(np.ascontiguousarray(Wq.T) * SWQ, -240.0, 240.0).astype(F8E4)
    wlr8 = np.clip(wlr[:512] * SWL, -240.0, 240.0).astype(F8E4)
    wlr16 = (wlr[512:] * (SX * SWL)).astype(BF16)

    bk = (W_slow_b[:DIM] * SKP).astype(np.float32)
    b_total = float(b_core * n_cores)
    bvcp = ((W_slow_b[DIM:2 * DIM] - W_fast_b) * (SAR / (b_total * SG))).astype(
        np.float32
    )
    bqs = (W_slow_b[2 * DIM:3 * DIM] - SHIFT).astype(np.float32)
    blr = np.ascontiguousarray(W_slow_b[3 * DIM:3 * DIM + 1]).astype(np.float32)
    wfb = np.ascontiguousarray(W_fast_b).astype(np.float32)

    in_maps = []
    for c in range(n_cores):
        xs = x[c * b_core:(c + 1) * b_core, :]
        xT = np.ascontiguousarray(xs.T)
        xT16 = np.ascontiguousarray(xT[512:]).astype(BF16)
        xT8 = np.clip(xT * SX, -240.0, 240.0).astype(F8E4)
        xn8 = np.clip(xs * SX, -240.0, 240.0).astype(F8E4)
        in_maps.append({
            "xT16": xT16, "xT8": xT8, "xn8": np.ascontiguousarray(xn8),
            "wk8": wk8, "wk16": wk16, "wq8": wq8, "wv16": wv16,
            "wlr8": wlr8, "wlr16": wlr16,
            "bk": bk, "bqs": bqs, "blr": blr, "bvcp": bvcp, "wfb": wfb,
        })
    return in_maps


_PROGRAM_CACHE = {}


def _get_program(b_core, n_cores=N_CORES):
    key = (b_core, n_cores)
    if key not in _PROGRAM_CACHE:
        _PROGRAM_CACHE[key] = _build_program(b_core, n_cores)
    return _PROGRAM_CACHE[key]


def _run_device(x, W_slow_w, W_slow_b, W_fast_b, trace=False):
    from concourse.bass_utils import run_bass_kernel_spmd

    b_core = x.shape[0] // N_CORES
    nc = _get_program(b_core)
    in_maps = _host_prep(x, W_slow_w, W_slow_b, W_fast_b, b_core, N_CORES)
    res = run_bass_kernel_spmd(nc, in_maps, list(range(N_CORES)), trace=trace)
    out = np.concatenate([res.results[c]["out"] for c in range(N_CORES)], axis=0)
    return out.astype(np.float32), res


def _reference_numpy(x, W_slow_w, W_slow_b, W_fast_w, W_fast_b):
    """Exact fallback (only used if W_fast_w != 0, which the spec never produces)."""
    x = x.astype(np.float64)
    s = x @ W_slow_w.astype(np.float64).T + W_slow_b.astype(np.float64)
    k = s[:, :DIM]
    v = s[:, DIM:2 * DIM]
    q = s[:, 2 * DIM:3 * DIM]
    lr = 1.0 / (1.0 + np.exp(-s[:, -1:]))
    ek = np.exp(k - k.max(axis=1, keepdims=True))
    ak = ek / ek.sum(axis=1, keepdims=True)
    v_bar = ak @ W_fast_w.astype(np.float64).T + W_fast_b.astype(np.float64)
    sigk = 1.0 / (1.0 + np.exp(-k))
    delta = (lr * (v - v_bar)).T @ sigk / x.shape[0]
    w_new = W_fast_w.astype(np.float64) + delta
    eq = np.exp(q - q.max(axis=1, keepdims=True))
    aq = eq / eq.sum(axis=1, keepdims=True)
    return (aq @ w_new.T + W_fast_b.astype(np.float64)).astype(np.float32)


def kernel(x, W_slow_w, W_slow_b, W_fast_w, W_fast_b):
    x = np.asarray(x)
    W_slow_w = np.asarray(W_slow_w)
    W_slow_b = np.asarray(W_slow_b)
    W_fast_w = np.asarray(W_fast_w)
    W_fast_b = np.asarray(W_fast_b)
    if np.any(W_fast_w):
        # Spec guarantees W_fast_w == 0; exact fallback for generality.
        return _reference_numpy(x, W_slow_w, W_slow_b, W_fast_w, W_fast_b)
    out, _ = _run_device(x, W_slow_w, W_slow_b, W_fast_b, trace=False)
    return out
